# revision 1
# baseline (speedup 1.0000x reference)
"""Trainium2 Bass kernel for nn_CrossAttention (16x6209x256 cross-attention).

Strategy
--------
Data-parallel over batch: 16 batches -> 8 cores x 2 batches. Each core runs an
identical Bass/Tile program on its own batch slice (pure SPMD, no collectives).

Per batch the math is
    mapped_a = a @ Wa + ba            [6209, 64]
    mapped_b = b @ Wb + bb            [256, 64]
    scores   = mapped_a @ mapped_b.T * 8
    attn     = softmax(scores, -1)
    out      = (attn @ mapped_b) @ Wc + bc

With no nonlinearity between the projections and the attention matmuls, the
small weights fold per batch (computed on device in exact fp32):
    Wfused    = 8 * Wa @ mapped_b.T               [256, 256]
    scoreBias = 8 * ba @ mapped_b.T               [256]
    Wout      = mapped_b @ Wc + 1 x bc            [256, 256]
    scores    = a @ Wfused + scoreBias
    out       = softmax(scores) @ Wout        (bias bc exact since rows sum to 1)

Precision: softmax amplifies score error by |scores| (~500 here), so the
scores matmul runs as a 3-term bf16 split (a = ahi+alo split on host,
Wfused = Whi+Wlo split on device): scores ~ ahi@Whi + alo@Whi + ahi@Wlo,
residual ~1e-4 absolute. Downstream matmuls use f32r (1.4e-4 relative,
harmless there). Weight prep runs in exact fp32.

Layout: input_a is transposed on host to [256, seq] so the contraction dim
arrives on SBUF partitions straight from DMA; output is produced transposed
[256, seq] and transposed back on host. attn is normalized in [i, j] layout
on DVE (per-partition 1/sumexp), transposed to [j, i] via PE transpose.
"""
import sys

for _p in ("/opt/trn_rl_repo",):
    if _p not in sys.path:
        sys.path.append(_p)

import numpy as np
import ml_dtypes

import concourse.bacc as bacc
import concourse.mybir as mybir
import concourse.tile as tile
from concourse.bass_utils import run_bass_kernel_spmd

F32 = mybir.dt.float32
F32R = mybir.dt.float32r
BF16 = mybir.dt.bfloat16
P = 128

N_CORES = 8
BATCHES_PER_CORE = 2
SEQ = 6209
DF = 256          # feature dim of a / b
HID = 64          # projection dim
DMA_MACRO = 2048  # rows fetched/stored per DMA instruction
CMACRO = 512      # rows per compute macro (4 subtiles of 128)


def _row_plan(n_rows):
    """[(dma_start, dma_len, [(cm_start_within_dma, cm_len), ...]), ...]"""
    plan = []
    pos = 0
    while pos < n_rows:
        d = min(DMA_MACRO, n_rows - pos)
        cms = []
        q = 0
        while q < d:
            c = min(CMACRO, d - q)
            cms.append((q, c))
            q += c
        plan.append((pos, d, cms))
        pos += d
    return plan


def build_program(seq=SEQ, batches=BATCHES_PER_CORE, use_ba=False):
    nc = bacc.Bacc("TRN2", target_bir_lowering=False, debug=False)

    a_hl = nc.dram_tensor("a_hl", [batches, 2 * DF, seq], BF16, kind="ExternalInput")
    b_t = nc.dram_tensor("b_t", [batches, DF, DF], F32, kind="ExternalInput")
    wat = nc.dram_tensor("wat", [HID, DF], F32, kind="ExternalInput")
    wb = nc.dram_tensor("wb", [DF, HID], F32, kind="ExternalInput")
    wc = nc.dram_tensor("wc", [HID, DF], F32, kind="ExternalInput")
    ba_d = nc.dram_tensor("ba_d", [HID, 1], F32, kind="ExternalInput")
    bb_d = nc.dram_tensor("bb_d", [HID, 1], F32, kind="ExternalInput")
    bc_d = nc.dram_tensor("bc_d", [1, DF], F32, kind="ExternalInput")
    eye_d = nc.dram_tensor("eye_d", [P, P], F32, kind="ExternalInput")
    ones_d = nc.dram_tensor("ones_d", [1, P], F32, kind="ExternalInput")
    out_t = nc.dram_tensor("out_t", [batches, DF, seq], F32, kind="ExternalOutput")

    Exp = mybir.ActivationFunctionType.Exp
    Copy = mybir.ActivationFunctionType.Copy
    Ident = mybir.ActivationFunctionType.Identity

    with tile.TileContext(nc) as tc:
        with (
            tc.tile_pool(name="const", bufs=1) as cpool,
            tc.tile_pool(name="wpool", bufs=2) as wpool,
            tc.tile_pool(name="apool", bufs=3) as apool,
            tc.tile_pool(name="mpool", bufs=2) as mpool,
            tc.tile_pool(name="opool", bufs=3) as opool,
            tc.tile_pool(name="pp", bufs=1, space="PSUM") as pp,
        ):
            # ---- per-core constants ----
            eye_sb = cpool.tile([P, P], F32)
            nc.sync.dma_start(eye_sb[:], eye_d[:])
            wat_sb = cpool.tile([HID, DF], F32)
            nc.sync.dma_start(wat_sb[:], wat[:])
            wb_sb = cpool.tile([P, 2, HID], F32)
            nc.sync.dma_start(wb_sb[:], wb[:].rearrange("(k p) h -> p k h", p=P))
            wc_sb = cpool.tile([HID, DF], F32)
            nc.sync.dma_start(wc_sb[:], wc[:])
            ba_sb = cpool.tile([HID, 1], F32)
            nc.sync.dma_start(ba_sb[:], ba_d[:])
            bb_sb = cpool.tile([HID, 1], F32)
            nc.sync.dma_start(bb_sb[:], bb_d[:])
            bc_sb = cpool.tile([1, DF], F32)
            nc.sync.dma_start(bc_sb[:], bc_d[:])
            ones_sb = cpool.tile([1, P], F32)
            nc.sync.dma_start(ones_sb[:], ones_d[:])

            for b in range(batches):
                # ---- per-batch fused weights (exact fp32 matmuls) ----
                bT_sb = wpool.tile([P, 2, DF], F32)
                nc.sync.dma_start(bT_sb[:], b_t[b].rearrange("(k p) j -> p k j", p=P))

                ps_mb = pp.tile([HID, DF], F32, tag="fin0")
                for k in range(2):
                    nc.tensor.matmul(
                        ps_mb[:],
                        wb_sb[:, k, :],
                        bT_sb[:, k, :],
                        start=(k == 0), stop=(k == 1),
                    )
                mapped_bT = wpool.tile([HID, DF], F32)
                nc.scalar.activation(mapped_bT[:], ps_mb[:], Ident, bias=bb_sb[:])

                # Wfused, split hi/lo into bf16 (scale 8 folded in)
                whi_sb = wpool.tile([P, 2, DF], BF16)
                wlo_sb = wpool.tile([P, 2, DF], BF16)
                for c in range(2):
                    ps_wf = pp.tile([P, DF], F32, tag="fin0")
                    nc.tensor.matmul(
                        ps_wf[:],
                        wat_sb[:, c * P:(c + 1) * P],
                        mapped_bT[:],
                        start=True, stop=True,
                    )
                    nc.scalar.activation(whi_sb[:, c, :], ps_wf[:], Copy, scale=8.0)
                    # wlo = 8*wf - whi (rounded to bf16)
                    nc.vector.scalar_tensor_tensor(
                        wlo_sb[:, c, :],
                        ps_wf[:],
                        8.0,
                        whi_sb[:, c, :],
                        op0=mybir.AluOpType.mult,
                        op1=mybir.AluOpType.subtract,
                    )

                if use_ba:
                    ps_sbias = pp.tile([1, DF], F32, tag="fin0")
                    nc.tensor.matmul(
                        ps_sbias[:],
                        ba_sb[:],
                        mapped_bT[:],
                        start=True, stop=True,
                    )
                    sbias_sb = wpool.tile([1, DF], F32)
                    nc.scalar.activation(sbias_sb[:], ps_sbias[:], Copy, scale=8.0)

                wo_sb = wpool.tile([P, 2, DF], F32R)
                for c in range(2):
                    ps_wo = pp.tile([P, DF], F32, tag="fin0")
                    nc.tensor.matmul(
                        ps_wo[:],
                        mapped_bT[:, c * P:(c + 1) * P],
                        wc_sb[:],
                        start=True, stop=False,
                    )
                    nc.tensor.matmul(
                        ps_wo[:],
                        ones_sb[:],
                        bc_sb[:],
                        start=False, stop=True,
                    )
                    nc.vector.tensor_copy(wo_sb[:, c, :], ps_wo[:])

                # ---- main loop ----
                for d0, dlen, cms in _row_plan(seq):
                    aT_sb = apool.tile([P, 4, DMA_MACRO], BF16, tag="aT")
                    nc.sync.dma_start(
                        aT_sb[:, :, :dlen],
                        a_hl[b][:, d0:d0 + dlen].rearrange(
                            "(k p) i -> p k i", p=P),
                    )
                    outT_sb = opool.tile([P, 2, DMA_MACRO], F32, tag="outT")

                    for mo, R in cms:
                        subs = [(o, min(P, R - o)) for o in range(0, R, P)]
                        ns = len(subs)

                        scores_ps = pp.tile([P, 4 * DF], F32, tag="scores", bufs=2)
                        for s, (io, r) in enumerate(subs):
                            c0 = s * DF
                            terms = []
                            for k in range(2):
                                ah = aT_sb[:, k, mo + io:mo + io + r]
                                al = aT_sb[:, 2 + k, mo + io:mo + io + r]
                                terms += [
                                    (ah, whi_sb[:, k, :]),
                                    (al, whi_sb[:, k, :]),
                                    (ah, wlo_sb[:, k, :]),
                                ]
                            for t, (lhs, rhs) in enumerate(terms):
                                nc.tensor.matmul(
                                    scores_ps[:r, c0:c0 + DF],
                                    lhs,
                                    rhs,
                                    start=(t == 0),
                                    stop=(t == len(terms) - 1) and not use_ba,
                                )
                            if use_ba:
                                nc.tensor.matmul(
                                    scores_ps[:r, c0:c0 + DF],
                                    ones_sb[:, :r],
                                    sbias_sb[:],
                                    start=False, stop=True,
                                )

                        rmax = max(r for _, r in subs)
                        negmax = mpool.tile([P, 4], F32, tag="negmax")
                        if all(r == rmax for _, r in subs):
                            nc.vector.tensor_reduce(
                                negmax[:rmax, :ns],
                                scores_ps[:rmax, :ns * DF].rearrange(
                                    "p (s j) -> p s j", s=ns),
                                axis=mybir.AxisListType.X,
                                op=mybir.AluOpType.max,
                                negate=True,
                            )
                        else:
                            for s, (io, r) in enumerate(subs):
                                nc.vector.tensor_reduce(
                                    negmax[:r, s:s + 1],
                                    scores_ps[:r, s * DF:(s + 1) * DF],
                                    axis=mybir.AxisListType.X,
                                    op=mybir.AluOpType.max,
                                    negate=True,
                                )

                        attn_sb = mpool.tile([P, 4 * DF], F32, tag="attn")
                        attn_n = mpool.tile([P, 4 * DF], F32, tag="attn_n")
                        sumexp = mpool.tile([P, 4], F32, tag="sumexp")
                        for s, (io, r) in enumerate(subs):
                            c0 = s * DF
                            nc.scalar.activation(
                                attn_sb[:r, c0:c0 + DF],
                                scores_ps[:r, c0:c0 + DF],
                                Exp,
                                bias=negmax[:r, s:s + 1],
                                accum_out=sumexp[:r, s:s + 1],
                            )
                        recip = mpool.tile([P, 4], F32, tag="recip")
                        if all(r == rmax for _, r in subs):
                            nc.vector.reciprocal(recip[:rmax, :ns], sumexp[:rmax, :ns])
                        else:
                            for s, (io, r) in enumerate(subs):
                                nc.vector.reciprocal(
                                    recip[:r, s:s + 1], sumexp[:r, s:s + 1])
                        for s, (io, r) in enumerate(subs):
                            c0 = s * DF
                            nc.vector.tensor_scalar_mul(
                                attn_n[:r, c0:c0 + DF],
                                attn_sb[:r, c0:c0 + DF],
                                recip[:r, s:s + 1],
                            )

                        aT0_ps = pp.tile([P, CMACRO], F32, tag="attnT0")
                        aT1_ps = pp.tile([P, CMACRO], F32, tag="attnT1")
                        for s, (io, r) in enumerate(subs):
                            c0 = s * DF
                            for jh, dst in ((0, aT0_ps), (1, aT1_ps)):
                                o_ap = dst[:, io:io + r]
                                i_ap = attn_n[:r, c0 + jh * P:c0 + (jh + 1) * P]
                                e_ap = eye_sb[:r, :r]
                                if r % 2:
                                    # f32r transpose needs an even moving dim
                                    o_ap = o_ap.bitcast(F32)
                                    i_ap = i_ap.bitcast(F32)
                                    e_ap = e_ap.bitcast(F32)
                                nc.tensor.transpose(o_ap, i_ap, e_ap)
                        attnT0 = mpool.tile([P, CMACRO], F32R, tag="attnT0sb")
                        attnT1 = mpool.tile([P, CMACRO], F32R, tag="attnT1sb")
                        nc.scalar.copy(attnT0[:, :R], aT0_ps[:, :R])
                        nc.vector.tensor_copy(attnT1[:, :R], aT1_ps[:, :R])

                        # final: outT[fo, i] = sum_j Wout[j, fo] attnT[j, i]
                        for c in range(2):
                            ps_fin = pp.tile([P, CMACRO], F32, tag=f"fin{c}")
                            for k, aTk in enumerate((attnT0, attnT1)):
                                # f32r needs an even moving dim; odd tails
                                # fall back to plain fp32 (tiny anyway)
                                if R % 2 == 0:
                                    lhs, rhs = (wo_sb[:, k, c * P:(c + 1) * P],
                                                aTk[:, :R])
                                else:
                                    lhs = wo_sb[:, k, c * P:(c + 1) * P].bitcast(F32)
                                    rhs = aTk[:, :R].bitcast(F32)
                                nc.tensor.matmul(
                                    ps_fin[:, :R],
                                    lhs,
                                    rhs,
                                    start=(k == 0), stop=(k == 1),
                                )
                            if c == 0:
                                nc.vector.tensor_copy(
                                    outT_sb[:, c, mo:mo + R], ps_fin[:, :R])
                            else:
                                nc.scalar.copy(
                                    outT_sb[:, c, mo:mo + R], ps_fin[:, :R])

                    nc.sync.dma_start(
                        out_t[b][:, d0:d0 + dlen].rearrange("(c p) i -> p c i", p=P),
                        outT_sb[:, :, :dlen],
                    )

    nc.compile()
    return nc


_PROGRAM_CACHE = {}


def _get_program(seq=SEQ, batches=BATCHES_PER_CORE, use_ba=False):
    key = (seq, batches, use_ba)
    if key not in _PROGRAM_CACHE:
        _PROGRAM_CACHE[key] = build_program(seq, batches, use_ba)
    return _PROGRAM_CACHE[key]


def make_in_maps(input_a, input_b, Wa, ba, Wb, bb, Wc, bc,
                 n_cores=N_CORES, batches=BATCHES_PER_CORE):
    input_a = np.asarray(input_a, dtype=np.float32)
    input_b = np.asarray(input_b, dtype=np.float32)
    a_t = np.ascontiguousarray(input_a.transpose(0, 2, 1))      # [B, DF, seq]
    a_hi = a_t.astype(ml_dtypes.bfloat16)
    a_lo = (a_t - a_hi.astype(np.float32)).astype(ml_dtypes.bfloat16)
    # rows 0..DF-1 = hi, DF..2DF-1 = lo  -> [B, 2*DF, seq]
    a_hl = np.ascontiguousarray(np.concatenate([a_hi, a_lo], axis=1))
    b_t = np.ascontiguousarray(input_b.transpose(0, 2, 1))
    shared = {
        "wat": np.ascontiguousarray(np.asarray(Wa, np.float32).T),
        "wb": np.ascontiguousarray(np.asarray(Wb, np.float32)),
        "wc": np.ascontiguousarray(np.asarray(Wc, np.float32)),
        "ba_d": np.asarray(ba, np.float32).reshape(HID, 1).copy(),
        "bb_d": np.asarray(bb, np.float32).reshape(HID, 1).copy(),
        "bc_d": np.asarray(bc, np.float32).reshape(1, DF).copy(),
        "eye_d": np.eye(P, dtype=np.float32),
        "ones_d": np.ones((1, P), dtype=np.float32),
    }
    in_maps = []
    for c in range(n_cores):
        lo, hi = c * batches, (c + 1) * batches
        in_maps.append({
            "a_hl": np.ascontiguousarray(a_hl[lo:hi]),
            "b_t": np.ascontiguousarray(b_t[lo:hi]),
            **shared,
        })
    return in_maps


def kernel(input_a, input_b, Wa, ba, Wb, bb, Wc, bc):
    use_ba = bool(np.any(np.asarray(ba)))
    nc = _get_program(use_ba=use_ba)
    in_maps = make_in_maps(input_a, input_b, Wa, ba, Wb, bb, Wc, bc)
    res = run_bass_kernel_spmd(nc, in_maps, core_ids=list(range(N_CORES)))
    outs = np.concatenate([r["out_t"] for r in res.results], axis=0)
    return np.ascontiguousarray(outs.transpose(0, 2, 1))



# revision 9
# speedup vs baseline: 1.0654x; 1.0654x over previous
"""Trainium2 Bass kernel for nn_CrossAttention (16x6209x256 cross-attention).

Strategy
--------
Data-parallel over batch: 16 batches -> 8 cores x 2 batches, pure SPMD.

Per batch:
    mapped_b = b @ Wb + bb                        [256, 64]
    Wfused   = 8 * Wa @ mapped_b.T                [256, 256]
    scores   = a @ Wfused (+ 8 * ba @ mapped_b.T)
    attnU    = exp(scores - max)                  (unnormalized)
    Wout     = mapped_b @ Wc + 1 x bc             [256, 256]
    out      = (attnU @ Wout) / sumexp            (divide on host; bc exact
                                                   because sum(attnU) = sumexp)

Precision: scores run at a 2^11 PSUM scale as
    fp16(a)*32 @ fp16(Wf)*64                      (fp16 hi term, 2 matmuls)
  + e4m3(alo*2^11) @ e4m3(Wf)                     } one fp8 DoubleRow pair
  + e4m3(a)       @ e4m3(Wlo*2^11)                } per k-chunk (2 matmuls)
where alo = a - fp16(a), Wlo = Wf - fp16(Wf). Exp then applies scale=2^-11.
Attn path (exp output, transpose, Wout, out) is all fp16: rel err ~1.7e-3.

Output is produced transposed [256, seq] in fp16 along with per-row sumexp;
host transposes, divides, and upcasts. Input a ships as fp16 + 2x fp8 planes
(4 B/elem total, same as fp32).
"""
import sys

for _p in ("/opt/trn_rl_repo",):
    if _p not in sys.path:
        sys.path.append(_p)

import numpy as np
import ml_dtypes

import concourse.bacc as bacc
import concourse.mybir as mybir
import concourse.tile as tile
from concourse.bass_utils import run_bass_kernel_spmd

F32 = mybir.dt.float32
F16 = mybir.dt.float16
F8 = mybir.dt.float8e4
P = 128

N_CORES = 8
BATCHES_PER_CORE = 2
SEQ = 6209
DF = 256          # feature dim of a / b
HID = 64          # projection dim
DMA_MACRO = 2048  # rows fetched/stored per DMA instruction
CMACRO = 512      # rows per compute macro (4 subtiles of 128)

SC = 2048.0       # 2^11 PSUM score scale
ISC = 1.0 / SC


def _row_plan(n_rows):
    """[(dma_start, dma_len, [(cm_start_within_dma, cm_len), ...]), ...]"""
    plan = []
    pos = 0
    while pos < n_rows:
        d = min(DMA_MACRO, n_rows - pos)
        cms = []
        q = 0
        while q < d:
            c = min(CMACRO, d - q)
            cms.append((q, c))
            q += c
        plan.append((pos, d, cms))
        pos += d
    return plan


def build_program(seq=SEQ, batches=BATCHES_PER_CORE, use_ba=False):
    nc = bacc.Bacc("TRN2", target_bir_lowering=False, debug=False)

    a16_d = nc.dram_tensor("a16_d", [batches, 2, P, seq], F16, kind="ExternalInput")
    a8_d = nc.dram_tensor("a8_d", [batches, P, 2, 2, seq], F8, kind="ExternalInput")
    b_t = nc.dram_tensor("b_t", [batches, DF, DF], F32, kind="ExternalInput")
    wat = nc.dram_tensor("wat", [HID, DF], F32, kind="ExternalInput")
    wb = nc.dram_tensor("wb", [DF, HID], F32, kind="ExternalInput")
    wc = nc.dram_tensor("wc", [HID, DF], F32, kind="ExternalInput")
    ba_d = nc.dram_tensor("ba_d", [HID, 1], F32, kind="ExternalInput")
    bb_d = nc.dram_tensor("bb_d", [HID, 1], F32, kind="ExternalInput")
    bc_d = nc.dram_tensor("bc_d", [1, DF], F32, kind="ExternalInput")
    eye_d = nc.dram_tensor("eye_d", [P, P], F16, kind="ExternalInput")
    ones_d = nc.dram_tensor("ones_d", [1, P], F32, kind="ExternalInput")
    out_t = nc.dram_tensor("out_t", [batches, DF, seq], F16, kind="ExternalOutput")
    n_sumcol = 4 * len([c for _, _, cs in _row_plan(seq) for c in cs])
    sum_d = nc.dram_tensor("sum_d", [batches, P, n_sumcol], F32,
                           kind="ExternalOutput")

    Exp = mybir.ActivationFunctionType.Exp
    Copy = mybir.ActivationFunctionType.Copy
    Ident = mybir.ActivationFunctionType.Identity
    DR = mybir.MatmulPerfMode.DoubleRow

    with tile.TileContext(nc) as tc:
        with (
            tc.tile_pool(name="const", bufs=1) as cpool,
            tc.tile_pool(name="wpool", bufs=2) as wpool,
            tc.tile_pool(name="apool", bufs=3) as apool,
            tc.tile_pool(name="mpool", bufs=2) as mpool,
            tc.tile_pool(name="opool", bufs=3) as opool,
            tc.tile_pool(name="pp", bufs=1, space="PSUM") as pp,
        ):
            # ---- per-core constants ----
            eye_sb = cpool.tile([P, P], F16)
            nc.sync.dma_start(eye_sb[:], eye_d[:])
            wat_sb = cpool.tile([HID, DF], F32)
            nc.sync.dma_start(wat_sb[:], wat[:])
            wb_sb = cpool.tile([P, 2, HID], F32)
            nc.sync.dma_start(wb_sb[:], wb[:].rearrange("(k p) h -> p k h", p=P))
            wc_sb = cpool.tile([HID, DF], F32)
            nc.sync.dma_start(wc_sb[:], wc[:])
            ba_sb = cpool.tile([HID, 1], F32)
            nc.sync.dma_start(ba_sb[:], ba_d[:])
            bb_sb = cpool.tile([HID, 1], F32)
            nc.sync.dma_start(bb_sb[:], bb_d[:])
            bc_sb = cpool.tile([1, DF], F32)
            nc.sync.dma_start(bc_sb[:], bc_d[:])
            ones_sb = cpool.tile([1, P], F32)
            nc.sync.dma_start(ones_sb[:], ones_d[:])

            for b in range(batches):
                # ---- per-batch fused weights (exact fp32 matmuls) ----
                bT_sb = wpool.tile([P, 2, DF], F32)
                nc.sync.dma_start(bT_sb[:], b_t[b].rearrange("(k p) j -> p k j", p=P))

                ps_mb = pp.tile([HID, DF], F32, tag="fin")
                for k in range(2):
                    nc.tensor.matmul(
                        ps_mb[:],
                        wb_sb[:, k, :],
                        bT_sb[:, k, :],
                        start=(k == 0), stop=(k == 1),
                    )
                mapped_bT = wpool.tile([HID, DF], F32)
                nc.scalar.activation(mapped_bT[:], ps_mb[:], Ident, bias=bb_sb[:])

                # Wfused splits: fp16 hi (x64 scale) + fp8 pair for DoubleRow
                w16s_sb = wpool.tile([P, 2, DF], F16)   # fp16(Wf)*64
                wpair_sb = wpool.tile([P, 2, 2, DF], F8)  # [e4m3(Wf) | e4m3(Wlo*2^11)]
                wf_sb = wpool.tile([P, 2, DF], F32)
                w16c_sb = wpool.tile([P, 2, DF], F16)
                wlo_sb = wpool.tile([P, 2, DF], F32)
                for c in range(2):
                    ps_wf = pp.tile([P, DF], F32, tag="fin")
                    nc.tensor.matmul(
                        ps_wf[:],
                        wat_sb[:, c * P:(c + 1) * P],
                        mapped_bT[:],
                        start=True, stop=True,
                    )
                    nc.scalar.activation(wf_sb[:, c, :], ps_wf[:], Copy, scale=8.0)
                    nc.scalar.activation(w16s_sb[:, c, :], ps_wf[:], Copy, scale=512.0)
                    nc.scalar.activation(w16c_sb[:, c, :], ps_wf[:], Copy, scale=8.0)
                    nc.scalar.activation(wpair_sb[:, c, 0, :], ps_wf[:], Copy,
                                         scale=8.0)
                    nc.vector.tensor_sub(wlo_sb[:, c, :], wf_sb[:, c, :],
                                         w16c_sb[:, c, :])
                    nc.scalar.activation(wpair_sb[:, c, 1, :], wlo_sb[:, c, :],
                                         Copy, scale=SC)

                if use_ba:
                    ps_sbias = pp.tile([1, DF], F32, tag="fin")
                    nc.tensor.matmul(
                        ps_sbias[:],
                        ba_sb[:],
                        mapped_bT[:],
                        start=True, stop=True,
                    )
                    sbias_sb = wpool.tile([1, DF], F32)
                    nc.scalar.activation(sbias_sb[:], ps_sbias[:], Copy,
                                         scale=8.0 * SC)

                # Wout = mapped_b @ Wc + 1 x bc, fp16, layout [j-part, jk, f]
                wo16_sb = wpool.tile([P, 2, DF], F16)
                for k in range(2):
                    ps_wo = pp.tile([P, DF], F32, tag="fin")
                    nc.tensor.matmul(
                        ps_wo[:],
                        mapped_bT[:, k * P:(k + 1) * P],
                        wc_sb[:],
                        start=True, stop=False,
                    )
                    nc.tensor.matmul(
                        ps_wo[:],
                        ones_sb[:],
                        bc_sb[:],
                        start=False, stop=True,
                    )
                    nc.scalar.activation(wo16_sb[:, k, :], ps_wo[:], Copy)

                sum_sb = wpool.tile([P, n_sumcol], F32, tag="sums")

                # ---- main loop ----
                gm = 0  # global cmacro index within the batch
                for d0, dlen, cms in _row_plan(seq):
                    a16_sb = apool.tile([P, 2, DMA_MACRO], F16, tag="a16")
                    nc.sync.dma_start(
                        a16_sb[:, :, :dlen],
                        a16_d[b][:, :, d0:d0 + dlen].rearrange("k p i -> p k i"),
                    )
                    a8_sb = apool.tile([P, 2, 2, DMA_MACRO], F8, tag="a8")
                    nc.sync.dma_start(
                        a8_sb[:, :, :, :dlen],
                        a8_d[b][:, :, :, d0:d0 + dlen],
                    )
                    outT_sb = opool.tile([P, 2, DMA_MACRO], F16, tag="outT")

                    for mo, R in cms:
                        subs = [(o, min(P, R - o)) for o in range(0, R, P)]
                        ns = len(subs)

                        scores_ps = pp.tile([P, 4 * DF], F32, tag="scores", bufs=2)
                        for s, (io, r) in enumerate(subs):
                            c0 = s * DF
                            go = mo + io
                            for k in range(2):
                                nc.tensor.matmul(
                                    scores_ps[:r, c0:c0 + DF],
                                    a16_sb[:, k, go:go + r],
                                    w16s_sb[:, k, :],
                                    start=(k == 0), stop=False,
                                )
                            for k in range(2):
                                nc.tensor.matmul(
                                    scores_ps[:r, c0:c0 + DF],
                                    a8_sb[:, k, :, go:go + r],
                                    wpair_sb[:, k, :, :],
                                    start=False,
                                    stop=(k == 1) and not use_ba,
                                    perf_mode=DR,
                                )
                            if use_ba:
                                nc.tensor.matmul(
                                    scores_ps[:r, c0:c0 + DF],
                                    ones_sb[:, :r],
                                    sbias_sb[:],
                                    start=False, stop=True,
                                )

                        rmax = max(r for _, r in subs)
                        req = all(r == rmax for _, r in subs)
                        negmax = mpool.tile([P, 4], F32, tag="negmax")
                        ebias = mpool.tile([P, 4], F32, tag="ebias")
                        if req:
                            nc.vector.tensor_reduce(
                                negmax[:rmax, :ns],
                                scores_ps[:rmax, :ns * DF].rearrange(
                                    "p (s j) -> p s j", s=ns),
                                axis=mybir.AxisListType.X,
                                op=mybir.AluOpType.max,
                                negate=True,
                            )
                            nc.gpsimd.tensor_scalar_mul(
                                ebias[:rmax, :ns], negmax[:rmax, :ns], ISC)
                        else:
                            for s, (io, r) in enumerate(subs):
                                nc.vector.tensor_reduce(
                                    negmax[:r, s:s + 1],
                                    scores_ps[:r, s * DF:(s + 1) * DF],
                                    axis=mybir.AxisListType.X,
                                    op=mybir.AluOpType.max,
                                    negate=True,
                                )
                                nc.gpsimd.tensor_scalar_mul(
                                    ebias[:r, s:s + 1], negmax[:r, s:s + 1], ISC)

                        attnU = mpool.tile([P, 4, DF], F16, tag="attnU")
                        for s, (io, r) in enumerate(subs):
                            nc.scalar.activation(
                                attnU[:r, s, :],
                                scores_ps[:r, s * DF:(s + 1) * DF],
                                Exp,
                                bias=ebias[:r, s:s + 1],
                                scale=ISC,
                            )

                        # sumexp of the rounded fp16 attn weights
                        if req:
                            nc.vector.tensor_reduce(
                                sum_sb[:rmax, 4 * gm:4 * gm + ns],
                                attnU[:rmax, :ns, :],
                                axis=mybir.AxisListType.X,
                                op=mybir.AluOpType.add,
                            )
                        else:
                            for s, (io, r) in enumerate(subs):
                                nc.vector.tensor_reduce(
                                    sum_sb[:r, 4 * gm + s:4 * gm + s + 1],
                                    attnU[:r, s, :],
                                    axis=mybir.AxisListType.X,
                                    op=mybir.AluOpType.add,
                                )

                        # transpose attnU -> [j, i] (fp16 PSUM)
                        aT_ps = pp.tile([P, 2, CMACRO], F16, tag="attnT", bufs=2)
                        for s, (io, r) in enumerate(subs):
                            rp = r + (r & 1)  # even moving dim for the PE
                            for jh in range(2):
                                nc.tensor.transpose(
                                    aT_ps[:, jh, io:io + rp],
                                    attnU[:rp, s, jh * P:(jh + 1) * P],
                                    eye_sb[:rp, :rp],
                                )
                        attnT = mpool.tile([P, 2, CMACRO], F16, tag="attnTsb")
                        nc.vector.tensor_copy(attnT[:, :, :R], aT_ps[:, :, :R])

                        # final: outT[f, i] = sum_j Wout[j, f] attnT[j, i]
                        fin_ps = pp.tile([P, 2, CMACRO], F32, tag="fin")
                        for c in range(2):
                            for k in range(2):
                                nc.tensor.matmul(
                                    fin_ps[:, c, :R],
                                    wo16_sb[:, k, c * P:(c + 1) * P],
                                    attnT[:, k, :R],
                                    start=(k == 0), stop=(k == 1),
                                )
                        nc.scalar.activation(
                            outT_sb[:, :, mo:mo + R], fin_ps[:, :, :R], Copy)
                        gm += 1

                    nc.sync.dma_start(
                        out_t[b][:, d0:d0 + dlen].rearrange("(c p) i -> p c i", p=P),
                        outT_sb[:, :, :dlen],
                    )
                nc.sync.dma_start(sum_d[b], sum_sb[:])

    nc.compile()
    return nc


_PROGRAM_CACHE = {}


def _get_program(seq=SEQ, batches=BATCHES_PER_CORE, use_ba=False):
    key = (seq, batches, use_ba)
    if key not in _PROGRAM_CACHE:
        _PROGRAM_CACHE[key] = build_program(seq, batches, use_ba)
    return _PROGRAM_CACHE[key]


def make_in_maps(input_a, input_b, Wa, ba, Wb, bb, Wc, bc,
                 n_cores=N_CORES, batches=BATCHES_PER_CORE):
    input_a = np.asarray(input_a, dtype=np.float32)
    input_b = np.asarray(input_b, dtype=np.float32)
    a_t = np.ascontiguousarray(input_a.transpose(0, 2, 1))      # [B, DF, seq]
    B, _, seq = a_t.shape
    a16 = a_t.astype(np.float16)
    a16s = (a16.astype(np.float32) * 32.0).astype(np.float16)
    alo8 = ((a_t - a16.astype(np.float32)) * SC).astype(ml_dtypes.float8_e4m3)
    a8 = a_t.astype(ml_dtypes.float8_e4m3)
    # [B, 128(f%128), 2(kchunk), 2(lo8|a8), seq]
    a8pair = np.empty((B, P, 2, 2, seq), dtype=ml_dtypes.float8_e4m3)
    a8pair[:, :, :, 0, :] = alo8.reshape(B, 2, P, seq).transpose(0, 2, 1, 3)
    a8pair[:, :, :, 1, :] = a8.reshape(B, 2, P, seq).transpose(0, 2, 1, 3)
    a16v = np.ascontiguousarray(a16s.reshape(B, 2, P, seq))
    b_t = np.ascontiguousarray(input_b.transpose(0, 2, 1))
    shared = {
        "wat": np.ascontiguousarray(np.asarray(Wa, np.float32).T),
        "wb": np.ascontiguousarray(np.asarray(Wb, np.float32)),
        "wc": np.ascontiguousarray(np.asarray(Wc, np.float32)),
        "ba_d": np.asarray(ba, np.float32).reshape(HID, 1).copy(),
        "bb_d": np.asarray(bb, np.float32).reshape(HID, 1).copy(),
        "bc_d": np.asarray(bc, np.float32).reshape(1, DF).copy(),
        "eye_d": np.eye(P, dtype=np.float16),
        "ones_d": np.ones((1, P), dtype=np.float32),
    }
    in_maps = []
    for c in range(n_cores):
        lo, hi = c * batches, (c + 1) * batches
        in_maps.append({
            "a16_d": np.ascontiguousarray(a16v[lo:hi]),
            "a8_d": np.ascontiguousarray(a8pair[lo:hi]),
            "b_t": np.ascontiguousarray(b_t[lo:hi]),
            **shared,
        })
    return in_maps


def _postprocess(results, seq=SEQ, batches=BATCHES_PER_CORE):
    """Concatenate per-core outputs, transpose, and normalize by sumexp."""
    outs = np.concatenate(
        [np.asarray(r["out_t"], dtype=np.float32) for r in results], axis=0)
    sums = np.concatenate(
        [np.asarray(r["sum_d"], dtype=np.float32) for r in results], axis=0)
    B = outs.shape[0]
    # sums[b, p, 4*gm + s] -> row i = 512*gm + 128*s + p
    n_cm = sums.shape[2] // 4
    grid = sums.reshape(B, P, n_cm, 4).transpose(0, 2, 3, 1).reshape(B, -1)
    se = grid[:, :seq]                                  # [B, seq]
    out = outs.transpose(0, 2, 1) / se[:, :, None]
    return np.ascontiguousarray(out.astype(np.float32))


def kernel(input_a, input_b, Wa, ba, Wb, bb, Wc, bc):
    use_ba = bool(np.any(np.asarray(ba)))
    nc = _get_program(use_ba=use_ba)
    in_maps = make_in_maps(input_a, input_b, Wa, ba, Wb, bb, Wc, bc)
    res = run_bass_kernel_spmd(nc, in_maps, core_ids=list(range(N_CORES)))
    return _postprocess(res.results)


# revision 13
# speedup vs baseline: 1.1336x; 1.0640x over previous
"""Trainium2 Bass kernel for nn_CrossAttention (16x6209x256 cross-attention).

Strategy
--------
Data-parallel over batch: 16 batches -> 8 cores x 2 batches, pure SPMD.

Per batch:
    mapped_b = b @ Wb + bb                        [256, 64]
    Wfused   = 8 * Wa @ mapped_b.T                [256, 256]
    scores   = a @ Wfused (+ 8 * ba @ mapped_b.T)
    attnU    = exp(scores - max)                  (unnormalized)
    Wout     = mapped_b @ Wc + 1 x bc             [256, 256]
    out      = (attnU @ Wout) / sumexp            (divide on host; bc exact
                                                   because sum(attnU) = sumexp)

Precision: scores run at a 2^11 PSUM scale as
    fp16(a)*32 @ fp16(Wf)*64                      (fp16 hi term, 2 matmuls)
  + e4m3(alo*2^11) @ e4m3(Wf)                     } one fp8 DoubleRow pair
  + e4m3(a)       @ e4m3(Wlo*2^11)                } per k-chunk (2 matmuls)
where alo = a - fp16(a), Wlo = Wf - fp16(Wf). Exp then applies scale=2^-11.
Attn path (exp output, transpose, Wout, out) is all fp16: rel err ~1.7e-3.

Output is produced transposed [256, seq] in fp16 along with per-row sumexp;
host transposes, divides, and upcasts. Input a ships as fp16 + 2x fp8 planes
(4 B/elem total, same as fp32).
"""
import sys

for _p in ("/opt/trn_rl_repo",):
    if _p not in sys.path:
        sys.path.append(_p)

import numpy as np
import ml_dtypes

import concourse.bacc as bacc
import concourse.mybir as mybir
import concourse.tile as tile
from concourse.bass_utils import run_bass_kernel_spmd

F32 = mybir.dt.float32
F16 = mybir.dt.float16
F8 = mybir.dt.float8e4
P = 128

N_CORES = 8
BATCHES_PER_CORE = 2
SEQ = 6209
DF = 256          # feature dim of a / b
HID = 64          # projection dim
DMA_MACRO = 2048  # rows fetched/stored per DMA instruction
CMACRO = 512      # rows per compute macro (4 subtiles of 128)

SC = 2048.0       # 2^11 PSUM score scale
ISC = 1.0 / SC


def _row_plan(n_rows):
    """[(dma_start, dma_len, [(cm_start_within_dma, cm_len), ...]), ...]"""
    plan = []
    pos = 0
    while pos < n_rows:
        d = min(DMA_MACRO, n_rows - pos)
        cms = []
        q = 0
        while q < d:
            c = min(CMACRO, d - q)
            cms.append((q, c))
            q += c
        plan.append((pos, d, cms))
        pos += d
    return plan


def _cmacro_list(seq, batches):
    """Flat list of compute-macro descriptors across both batches.

    Each entry: dict(b, d0, dlen, mo, R, chunk_id, first_in_chunk,
                     last_in_chunk, first_in_batch, last_in_batch, gm)
    """
    out = []
    for b in range(batches):
        plan = _row_plan(seq)
        gm = 0
        for ci, (d0, dlen, cms) in enumerate(plan):
            for mi, (mo, R) in enumerate(cms):
                out.append(dict(
                    b=b, d0=d0, dlen=dlen, mo=mo, R=R,
                    chunk=(b, ci), n_chunks=len(plan), ci=ci,
                    first_in_chunk=(mi == 0), last_in_chunk=(mi == len(cms) - 1),
                    first_in_batch=(ci == 0 and mi == 0),
                    last_in_batch=(ci == len(plan) - 1 and mi == len(cms) - 1),
                    gm=gm,
                ))
                gm += 1
    return out


def build_program(seq=SEQ, batches=BATCHES_PER_CORE, use_ba=False):
    nc = bacc.Bacc("TRN2", target_bir_lowering=False, debug=False)

    a16_d = nc.dram_tensor("a16_d", [batches, 2, P, seq], F16, kind="ExternalInput")
    a8_d = nc.dram_tensor("a8_d", [batches, P, 2, 2, seq], F8, kind="ExternalInput")
    b_t = nc.dram_tensor("b_t", [batches, DF, DF], F32, kind="ExternalInput")
    wat = nc.dram_tensor("wat", [HID, DF], F32, kind="ExternalInput")
    wb = nc.dram_tensor("wb", [DF, HID], F32, kind="ExternalInput")
    wc = nc.dram_tensor("wc", [HID, DF], F32, kind="ExternalInput")
    ba_d = nc.dram_tensor("ba_d", [HID, 1], F32, kind="ExternalInput")
    bb_d = nc.dram_tensor("bb_d", [HID, 1], F32, kind="ExternalInput")
    bc_d = nc.dram_tensor("bc_d", [1, DF], F32, kind="ExternalInput")
    eye_d = nc.dram_tensor("eye_d", [P, P], F16, kind="ExternalInput")
    ones_d = nc.dram_tensor("ones_d", [1, P], F32, kind="ExternalInput")
    out_t = nc.dram_tensor("out_t", [batches, DF, seq], F16, kind="ExternalOutput")
    n_sumcol = 4 * len([c for _, _, cs in _row_plan(seq) for c in cs])
    sum_d = nc.dram_tensor("sum_d", [batches, P, n_sumcol], F16,
                           kind="ExternalOutput")

    Exp = mybir.ActivationFunctionType.Exp
    Copy = mybir.ActivationFunctionType.Copy
    Ident = mybir.ActivationFunctionType.Identity
    DR = mybir.MatmulPerfMode.DoubleRow

    with tile.TileContext(nc) as tc:
        with (
            tc.tile_pool(name="const", bufs=1) as cpool,
            tc.tile_pool(name="wpool", bufs=2) as wpool,
            tc.tile_pool(name="apool", bufs=3) as apool,
            tc.tile_pool(name="mpool", bufs=2) as mpool,
            tc.tile_pool(name="opool", bufs=3) as opool,
            tc.tile_pool(name="pp", bufs=1, space="PSUM") as pp,
        ):
            # ---- per-core constants ----
            eye_sb = cpool.tile([P, P], F16)
            nc.sync.dma_start(eye_sb[:], eye_d[:])
            wat_sb = cpool.tile([HID, DF], F32)
            nc.sync.dma_start(wat_sb[:], wat[:])
            wb_sb = cpool.tile([P, 2, HID], F32)
            nc.sync.dma_start(wb_sb[:], wb[:].rearrange("(k p) h -> p k h", p=P))
            wc_sb = cpool.tile([HID, DF], F32)
            nc.sync.dma_start(wc_sb[:], wc[:])
            ba_sb = cpool.tile([HID, 1], F32)
            nc.sync.dma_start(ba_sb[:], ba_d[:])
            bb_sb = cpool.tile([HID, 1], F32)
            nc.sync.dma_start(bb_sb[:], bb_d[:])
            bc_sb = cpool.tile([1, DF], F32)
            nc.sync.dma_start(bc_sb[:], bc_d[:])
            ones_sb = cpool.tile([1, P], F32)
            nc.sync.dma_start(ones_sb[:], ones_d[:])

            cmac = _cmacro_list(seq, batches)
            M = len(cmac)
            W = [None] * batches    # per-batch weight tiles
            ain = {}                # chunk -> (a16_sb, a8_sb)
            outbuf = {}             # chunk -> outT_sb
            ctx = [None] * M        # per-cmacro live tiles

            def prep_weights(b):
                bT_sb = wpool.tile([P, 2, DF], F32, tag="bT")
                nc.sync.dma_start(bT_sb[:], b_t[b].rearrange("(k p) j -> p k j", p=P))
                ps_mb = pp.tile([HID, DF], F32, tag="scores", bufs=2)
                for k in range(2):
                    nc.tensor.matmul(ps_mb[:], wb_sb[:, k, :], bT_sb[:, k, :],
                                     start=(k == 0), stop=(k == 1))
                mapped_bT = wpool.tile([HID, DF], F32, tag="mbT")
                nc.scalar.activation(mapped_bT[:], ps_mb[:], Ident, bias=bb_sb[:])

                w16s_sb = wpool.tile([P, 2, DF], F16, tag="w16s")
                wpair_sb = wpool.tile([P, 2, 2, DF], F8, tag="wpair")
                wf_sb = wpool.tile([P, 2, DF], F32, tag="wf")
                w16c_sb = wpool.tile([P, 2, DF], F16, tag="w16c")
                wlo_sb = wpool.tile([P, 2, DF], F32, tag="wlo")
                for c in range(2):
                    ps_wf = pp.tile([P, DF], F32, tag="scores", bufs=2)
                    nc.tensor.matmul(ps_wf[:], wat_sb[:, c * P:(c + 1) * P],
                                     mapped_bT[:], start=True, stop=True)
                    nc.scalar.activation(wf_sb[:, c, :], ps_wf[:], Copy, scale=8.0)
                    nc.scalar.activation(w16s_sb[:, c, :], ps_wf[:], Copy,
                                         scale=512.0)
                    nc.scalar.activation(w16c_sb[:, c, :], ps_wf[:], Copy, scale=8.0)
                    nc.scalar.activation(wpair_sb[:, c, 0, :], ps_wf[:], Copy,
                                         scale=8.0)
                    nc.vector.tensor_sub(wlo_sb[:, c, :], wf_sb[:, c, :],
                                         w16c_sb[:, c, :])
                    nc.scalar.activation(wpair_sb[:, c, 1, :], wlo_sb[:, c, :],
                                         Copy, scale=SC)

                sbias_sb = None
                if use_ba:
                    ps_sbias = pp.tile([1, DF], F32, tag="scores", bufs=2)
                    nc.tensor.matmul(ps_sbias[:], ba_sb[:], mapped_bT[:],
                                     start=True, stop=True)
                    sbias_sb = wpool.tile([1, DF], F32, tag="sbias")
                    nc.scalar.activation(sbias_sb[:], ps_sbias[:], Copy,
                                         scale=8.0 * SC)

                wo16_sb = wpool.tile([P, 2, DF], F16, tag="wo16")
                for k in range(2):
                    ps_wo = pp.tile([P, DF], F32, tag="scores", bufs=2)
                    nc.tensor.matmul(ps_wo[:], mapped_bT[:, k * P:(k + 1) * P],
                                     wc_sb[:], start=True, stop=False)
                    nc.tensor.matmul(ps_wo[:], ones_sb[:], bc_sb[:],
                                     start=False, stop=True)
                    nc.scalar.activation(wo16_sb[:, k, :], ps_wo[:], Copy)

                sum_sb = wpool.tile([P, n_sumcol], F16, tag="sums")
                return dict(w16s=w16s_sb, wpair=wpair_sb, wo16=wo16_sb,
                            sbias=sbias_sb, sums=sum_sb)

            def fetch_chunk(cm):
                key = cm["chunk"]
                if key in ain:
                    return
                b, d0, dlen = cm["b"], cm["d0"], cm["dlen"]
                a16_sb = apool.tile([P, 2, DMA_MACRO], F16, tag="a16")
                nc.sync.dma_start(
                    a16_sb[:, :, :dlen],
                    a16_d[b][:, :, d0:d0 + dlen].rearrange("k p i -> p k i"),
                )
                a8_sb = apool.tile([P, 2, 2, DMA_MACRO], F8, tag="a8")
                nc.sync.dma_start(
                    a8_sb[:, :, :, :dlen],
                    a8_d[b][:, :, :, d0:d0 + dlen],
                )
                ain[key] = (a16_sb, a8_sb)

            def subs_of(cm):
                return [(o, min(P, cm["R"] - o)) for o in range(0, cm["R"], P)]

            def stage_scores(m):
                cm = cmac[m]
                if cm["first_in_batch"]:
                    W[cm["b"]] = prep_weights(cm["b"])
                fetch_chunk(cm)
                # prefetch the next chunk (possibly next batch)
                nxt = next((c for c in cmac[m + 1:] if c["chunk"] != cm["chunk"]),
                           None)
                if nxt is not None:
                    fetch_chunk(nxt)
                w = W[cm["b"]]
                a16_sb, a8_sb = ain[cm["chunk"]]
                scores_ps = pp.tile([P, 4 * DF], F32, tag="scores", bufs=2)
                for s, (io, r) in enumerate(subs_of(cm)):
                    c0 = s * DF
                    go = cm["mo"] + io
                    for k in range(2):
                        nc.tensor.matmul(
                            scores_ps[:r, c0:c0 + DF],
                            a16_sb[:, k, go:go + r],
                            w["w16s"][:, k, :],
                            start=(k == 0), stop=False,
                        )
                    for k in range(2):
                        nc.tensor.matmul(
                            scores_ps[:r, c0:c0 + DF],
                            a8_sb[:, k, :, go:go + r],
                            w["wpair"][:, k, :, :],
                            start=False,
                            stop=(k == 1) and not use_ba,
                            perf_mode=DR,
                        )
                    if use_ba:
                        nc.tensor.matmul(
                            scores_ps[:r, c0:c0 + DF],
                            ones_sb[:, :r],
                            w["sbias"][:],
                            start=False, stop=True,
                        )
                ctx[m] = dict(scores=scores_ps)

            def stage_stats(m):
                cm = cmac[m]
                subs = subs_of(cm)
                ns = len(subs)
                scores_ps = ctx[m]["scores"]
                rmax = max(r for _, r in subs)
                req = all(r == rmax for _, r in subs)
                negmax = mpool.tile([P, 4], F32, tag="negmax")
                ebias = mpool.tile([P, 4], F32, tag="ebias")
                if req:
                    nc.vector.tensor_reduce(
                        negmax[:rmax, :ns],
                        scores_ps[:rmax, :ns * DF].rearrange(
                            "p (s j) -> p s j", s=ns),
                        axis=mybir.AxisListType.X,
                        op=mybir.AluOpType.max,
                        negate=True,
                    )
                    nc.gpsimd.tensor_scalar_mul(
                        ebias[:rmax, :ns], negmax[:rmax, :ns], ISC)
                else:
                    for s, (io, r) in enumerate(subs):
                        nc.vector.tensor_reduce(
                            negmax[:r, s:s + 1],
                            scores_ps[:r, s * DF:(s + 1) * DF],
                            axis=mybir.AxisListType.X,
                            op=mybir.AluOpType.max,
                            negate=True,
                        )
                        nc.gpsimd.tensor_scalar_mul(
                            ebias[:r, s:s + 1], negmax[:r, s:s + 1], ISC)
                attnU = mpool.tile([P, 4, DF], F16, tag="attnU")
                for s, (io, r) in enumerate(subs):
                    nc.scalar.activation(
                        attnU[:r, s, :],
                        scores_ps[:r, s * DF:(s + 1) * DF],
                        Exp,
                        bias=ebias[:r, s:s + 1],
                        scale=ISC,
                    )
                ctx[m]["attnU"] = attnU

            def stage_transp(m):
                cm = cmac[m]
                attnU = ctx[m]["attnU"]
                aT_ps = pp.tile([P, 2, CMACRO], F16, tag="attnT", bufs=2)
                for s, (io, r) in enumerate(subs_of(cm)):
                    rp = r + (r & 1)  # even moving dim for the PE
                    for jh in range(2):
                        nc.tensor.transpose(
                            aT_ps[:, jh, io:io + rp],
                            attnU[:rp, s, jh * P:(jh + 1) * P],
                            eye_sb[:rp, :rp],
                        )
                ctx[m]["aT_ps"] = aT_ps

            def stage_attnT_sum(m):
                cm = cmac[m]
                subs = subs_of(cm)
                ns = len(subs)
                gm = cm["gm"]
                attnU = ctx[m]["attnU"]
                sum_sb = W[cm["b"]]["sums"]
                rmax = max(r for _, r in subs)
                req = all(r == rmax for _, r in subs)
                with nc.allow_low_precision("fp16 sumexp is plenty (<=2^-11 rel)"):
                    if req:
                        nc.vector.tensor_reduce(
                            sum_sb[:rmax, 4 * gm:4 * gm + ns],
                            attnU[:rmax, :ns, :],
                            axis=mybir.AxisListType.X,
                            op=mybir.AluOpType.add,
                        )
                    else:
                        for s, (io, r) in enumerate(subs):
                            nc.vector.tensor_reduce(
                                sum_sb[:r, 4 * gm + s:4 * gm + s + 1],
                                attnU[:r, s, :],
                                axis=mybir.AxisListType.X,
                                op=mybir.AluOpType.add,
                            )
                attnT = mpool.tile([P, 2, CMACRO], F16, tag="attnTsb")
                nc.vector.tensor_copy(attnT[:, :, :cm["R"]],
                                      ctx[m]["aT_ps"][:, :, :cm["R"]])
                ctx[m]["attnT"] = attnT
                if cm["last_in_batch"]:
                    nc.sync.dma_start(sum_d[cm["b"]], sum_sb[:])

            def stage_final(m):
                cm = cmac[m]
                R = cm["R"]
                attnT = ctx[m]["attnT"]
                w = W[cm["b"]]
                if cm["first_in_chunk"]:
                    outT_sb = opool.tile([P, 2, DMA_MACRO], F16, tag="outT")
                    outbuf[cm["chunk"]] = outT_sb
                fin_ps = pp.tile([P, 2, CMACRO], F32, tag="fin", bufs=1)
                for c in range(2):
                    for k in range(2):
                        nc.tensor.matmul(
                            fin_ps[:, c, :R],
                            w["wo16"][:, k, c * P:(c + 1) * P],
                            attnT[:, k, :R],
                            start=(k == 0), stop=(k == 1),
                        )
                ctx[m]["fin"] = fin_ps

            def stage_outT(m):
                cm = cmac[m]
                outT_sb = outbuf[cm["chunk"]]
                nc.scalar.activation(
                    outT_sb[:, :, cm["mo"]:cm["mo"] + cm["R"]],
                    ctx[m]["fin"][:, :, :cm["R"]], Copy)
                if cm["last_in_chunk"]:
                    b, d0, dlen = cm["b"], cm["d0"], cm["dlen"]
                    nc.sync.dma_start(
                        out_t[b][:, d0:d0 + dlen].rearrange(
                            "(c p) i -> p c i", p=P),
                        outT_sb[:, :, :dlen],
                    )
                ctx[m] = None  # release references

            # ---- software-pipelined emission ----
            for m in range(M + 2):
                if m < M:
                    stage_scores(m)
                if 2 <= m and m - 2 < M:
                    stage_final(m - 2)
                if 1 <= m and m - 1 < M:
                    stage_transp(m - 1)
                if 2 <= m and m - 2 < M:
                    stage_outT(m - 2)
                if 1 <= m and m - 1 < M:
                    stage_attnT_sum(m - 1)
                if m < M:
                    stage_stats(m)

    nc.compile()
    return nc


_PROGRAM_CACHE = {}


def _get_program(seq=SEQ, batches=BATCHES_PER_CORE, use_ba=False):
    key = (seq, batches, use_ba)
    if key not in _PROGRAM_CACHE:
        _PROGRAM_CACHE[key] = build_program(seq, batches, use_ba)
    return _PROGRAM_CACHE[key]


def make_in_maps(input_a, input_b, Wa, ba, Wb, bb, Wc, bc,
                 n_cores=N_CORES, batches=BATCHES_PER_CORE):
    input_a = np.asarray(input_a, dtype=np.float32)
    input_b = np.asarray(input_b, dtype=np.float32)
    a_t = np.ascontiguousarray(input_a.transpose(0, 2, 1))      # [B, DF, seq]
    B, _, seq = a_t.shape
    a16 = a_t.astype(np.float16)
    a16s = (a16.astype(np.float32) * 32.0).astype(np.float16)
    alo8 = ((a_t - a16.astype(np.float32)) * SC).astype(ml_dtypes.float8_e4m3)
    a8 = a_t.astype(ml_dtypes.float8_e4m3)
    # [B, 128(f%128), 2(kchunk), 2(lo8|a8), seq]
    a8pair = np.empty((B, P, 2, 2, seq), dtype=ml_dtypes.float8_e4m3)
    a8pair[:, :, :, 0, :] = alo8.reshape(B, 2, P, seq).transpose(0, 2, 1, 3)
    a8pair[:, :, :, 1, :] = a8.reshape(B, 2, P, seq).transpose(0, 2, 1, 3)
    a16v = np.ascontiguousarray(a16s.reshape(B, 2, P, seq))
    b_t = np.ascontiguousarray(input_b.transpose(0, 2, 1))
    shared = {
        "wat": np.ascontiguousarray(np.asarray(Wa, np.float32).T),
        "wb": np.ascontiguousarray(np.asarray(Wb, np.float32)),
        "wc": np.ascontiguousarray(np.asarray(Wc, np.float32)),
        "ba_d": np.asarray(ba, np.float32).reshape(HID, 1).copy(),
        "bb_d": np.asarray(bb, np.float32).reshape(HID, 1).copy(),
        "bc_d": np.asarray(bc, np.float32).reshape(1, DF).copy(),
        "eye_d": np.eye(P, dtype=np.float16),
        "ones_d": np.ones((1, P), dtype=np.float32),
    }
    in_maps = []
    for c in range(n_cores):
        lo, hi = c * batches, (c + 1) * batches
        in_maps.append({
            "a16_d": np.ascontiguousarray(a16v[lo:hi]),
            "a8_d": np.ascontiguousarray(a8pair[lo:hi]),
            "b_t": np.ascontiguousarray(b_t[lo:hi]),
            **shared,
        })
    return in_maps


def _postprocess(results, seq=SEQ, batches=BATCHES_PER_CORE):
    """Concatenate per-core outputs, transpose, and normalize by sumexp."""
    outs = np.concatenate(
        [np.asarray(r["out_t"], dtype=np.float32) for r in results], axis=0)
    sums = np.concatenate(
        [np.asarray(r["sum_d"], dtype=np.float32) for r in results], axis=0)
    B = outs.shape[0]
    # sums[b, p, 4*gm + s] -> row i = 512*gm + 128*s + p
    n_cm = sums.shape[2] // 4
    grid = sums.reshape(B, P, n_cm, 4).transpose(0, 2, 3, 1).reshape(B, -1)
    se = grid[:, :seq]                                  # [B, seq]
    out = outs.transpose(0, 2, 1) / se[:, :, None]
    return np.ascontiguousarray(out.astype(np.float32))


def kernel(input_a, input_b, Wa, ba, Wb, bb, Wc, bc):
    use_ba = bool(np.any(np.asarray(ba)))
    nc = _get_program(use_ba=use_ba)
    in_maps = make_in_maps(input_a, input_b, Wa, ba, Wb, bb, Wc, bc)
    res = run_bass_kernel_spmd(nc, in_maps, core_ids=list(range(N_CORES)))
    return _postprocess(res.results)


# revision 15
# speedup vs baseline: 1.2442x; 1.0976x over previous
"""Trainium2 Bass kernel for nn_CrossAttention (16x6209x256 cross-attention).

Strategy
--------
Data-parallel over batch: 16 batches -> 8 cores x 2 batches, pure SPMD.

All weight prep happens on the HOST (tiny matmuls, exact fp32):
    mapped_b = b @ Wb + bb                        [256, 64]
    Wf       = 8 * Wa @ mapped_b.T                [256, 256]
    Wout     = mapped_b @ Wc + 1 x bc             [256, 256]
The device computes, per batch:
    scores = a @ Wf  (at a 2^11 PSUM scale) as
        fp16(a)*32 @ fp16(Wf)*64                  (fp16 hi term, 2 matmuls)
      + e4m3(alo*2^11) @ e4m3(Wf)                 } one fp8 DoubleRow pair
      + e4m3(a)        @ e4m3(Wlo*2^11)           } per k-chunk
    attnU  = exp(scores*2^-11 - max)              fp16 (unnormalized)
    sumexp = sum_j attnU                          (DVE reduce, fp16)
    outT   = Wout^T @ attnU^T                     (PE transpose + fp16 matmul)
Host postprocess: out = outT.T / sumexp (+exact bc since sum(attnU)=sumexp).
rel err ~1.7e-3 (validated in simulation against the fp32 reference).

The main loop is software-pipelined: per iteration m the PE stream is
scores(m), final(m-2), transp(m-1) so every cross-engine dependency has a
full iteration of slack; DVE runs sumexp(m-1), attnT copy(m-1), reduce(m);
ACT runs exp(m) then outT(m-2).
"""
import sys

for _p in ("/opt/trn_rl_repo",):
    if _p not in sys.path:
        sys.path.append(_p)

import numpy as np
import ml_dtypes

import concourse.bacc as bacc
import concourse.mybir as mybir
import concourse.tile as tile
from concourse.bass_utils import run_bass_kernel_spmd

F32 = mybir.dt.float32
F16 = mybir.dt.float16
F8 = mybir.dt.float8e4
P = 128

N_CORES = 8
BATCHES_PER_CORE = 2
SEQ = 6209
DF = 256          # feature dim of a / b
HID = 64          # projection dim
DMA_MACRO = 2048  # rows fetched/stored per DMA instruction
CMACRO = 512      # rows per compute macro (4 subtiles of 128)

SC = 2048.0       # 2^11 PSUM score scale
ISC = 1.0 / SC


def _row_plan(n_rows):
    """[(dma_start, dma_len, [(cm_start_within_dma, cm_len), ...]), ...]"""
    plan = []
    pos = 0
    while pos < n_rows:
        d = min(DMA_MACRO, n_rows - pos)
        cms = []
        q = 0
        while q < d:
            c = min(CMACRO, d - q)
            cms.append((q, c))
            q += c
        plan.append((pos, d, cms))
        pos += d
    return plan


def _cmacro_list(seq, batches):
    out = []
    for b in range(batches):
        plan = _row_plan(seq)
        gm = 0
        for ci, (d0, dlen, cms) in enumerate(plan):
            for mi, (mo, R) in enumerate(cms):
                out.append(dict(
                    b=b, d0=d0, dlen=dlen, mo=mo, R=R,
                    chunk=(b, ci),
                    first_in_chunk=(mi == 0), last_in_chunk=(mi == len(cms) - 1),
                    last_in_batch=(ci == len(plan) - 1 and mi == len(cms) - 1),
                    gm=gm,
                ))
                gm += 1
    return out


def build_program(seq=SEQ, batches=BATCHES_PER_CORE, use_ba=False):
    nc = bacc.Bacc("TRN2", target_bir_lowering=False, debug=False)

    a16_d = nc.dram_tensor("a16_d", [batches, 2, P, seq], F16, kind="ExternalInput")
    a8_d = nc.dram_tensor("a8_d", [batches, P, 2, 2, seq], F8, kind="ExternalInput")
    w16_d = nc.dram_tensor("w16_d", [batches, P, 2, DF], F16, kind="ExternalInput")
    w8_d = nc.dram_tensor("w8_d", [batches, P, 2, 2, DF], F8, kind="ExternalInput")
    wo_d = nc.dram_tensor("wo_d", [batches, P, 2, DF], F16, kind="ExternalInput")
    sb_d = nc.dram_tensor("sb_d", [batches, 1, DF], F32, kind="ExternalInput")
    eye_d = nc.dram_tensor("eye_d", [P, P], F16, kind="ExternalInput")
    ones_d = nc.dram_tensor("ones_d", [1, P], F32, kind="ExternalInput")
    out_t = nc.dram_tensor("out_t", [batches, DF, seq], F16, kind="ExternalOutput")
    n_sumcol = 4 * len([c for _, _, cs in _row_plan(seq) for c in cs])
    sum_d = nc.dram_tensor("sum_d", [batches, P, n_sumcol], F16,
                           kind="ExternalOutput")

    Exp = mybir.ActivationFunctionType.Exp
    Copy = mybir.ActivationFunctionType.Copy
    DR = mybir.MatmulPerfMode.DoubleRow

    with tile.TileContext(nc) as tc:
        with (
            tc.tile_pool(name="const", bufs=1) as cpool,
            tc.tile_pool(name="apool", bufs=3) as apool,
            tc.tile_pool(name="mpool", bufs=2) as mpool,
            tc.tile_pool(name="opool", bufs=3) as opool,
            tc.tile_pool(name="pp", bufs=1, space="PSUM") as pp,
        ):
            # ---- constants and per-batch weights (host-prepped) ----
            eye_sb = cpool.tile([P, P], F16)
            nc.sync.dma_start(eye_sb[:], eye_d[:])
            ones_sb = cpool.tile([1, P], F32)
            nc.sync.dma_start(ones_sb[:], ones_d[:])
            w16_sb = cpool.tile([P, batches, 2, DF], F16)
            nc.sync.dma_start(w16_sb[:], w16_d[:].rearrange("b p k j -> p b k j"))
            w8_sb = cpool.tile([P, batches, 2, 2, DF], F8)
            nc.sync.dma_start(w8_sb[:], w8_d[:].rearrange("b p k t j -> p b k t j"))
            wo_sb = cpool.tile([P, batches, 2, DF], F16)
            nc.sync.dma_start(wo_sb[:], wo_d[:].rearrange("b p k f -> p b k f"))
            sbias_sb = cpool.tile([1, batches, DF], F32)
            nc.sync.dma_start(sbias_sb[:], sb_d[:].rearrange("b x j -> x b j"))

            cmac = _cmacro_list(seq, batches)
            M = len(cmac)
            sums = [None] * batches
            ain = {}
            outbuf = {}
            ctx = [None] * M

            def fetch_chunk(cm):
                key = cm["chunk"]
                if key in ain:
                    return
                b, d0, dlen = cm["b"], cm["d0"], cm["dlen"]
                a16_sb = apool.tile([P, 2, DMA_MACRO], F16, tag="a16")
                nc.sync.dma_start(
                    a16_sb[:, :, :dlen],
                    a16_d[b][:, :, d0:d0 + dlen].rearrange("k p i -> p k i"),
                )
                a8_sb = apool.tile([P, 2, 2, DMA_MACRO], F8, tag="a8")
                nc.sync.dma_start(
                    a8_sb[:, :, :, :dlen],
                    a8_d[b][:, :, :, d0:d0 + dlen],
                )
                ain[key] = (a16_sb, a8_sb)

            def subs_of(cm):
                return [(o, min(P, cm["R"] - o)) for o in range(0, cm["R"], P)]

            def stage_scores(m):
                cm = cmac[m]
                b = cm["b"]
                if cm["gm"] == 0 and sums[b] is None:
                    sum_sb = mpool.tile([P, n_sumcol], F16, tag=f"sums{b}", bufs=1)
                    sums[b] = sum_sb
                fetch_chunk(cm)
                nxt = next((c for c in cmac[m + 1:] if c["chunk"] != cm["chunk"]),
                           None)
                if nxt is not None:
                    fetch_chunk(nxt)
                a16_sb, a8_sb = ain[cm["chunk"]]
                scores_ps = pp.tile([P, 4 * DF], F32, tag="scores", bufs=2)
                for s, (io, r) in enumerate(subs_of(cm)):
                    c0 = s * DF
                    go = cm["mo"] + io
                    for k in range(2):
                        nc.tensor.matmul(
                            scores_ps[:r, c0:c0 + DF],
                            a16_sb[:, k, go:go + r],
                            w16_sb[:, b, k, :],
                            start=(k == 0), stop=False,
                        )
                    for k in range(2):
                        nc.tensor.matmul(
                            scores_ps[:r, c0:c0 + DF],
                            a8_sb[:, k, :, go:go + r],
                            w8_sb[:, b, k, :, :],
                            start=False,
                            stop=(k == 1) and not use_ba,
                            perf_mode=DR,
                        )
                    if use_ba:
                        nc.tensor.matmul(
                            scores_ps[:r, c0:c0 + DF],
                            ones_sb[:, :r],
                            sbias_sb[:, b, :],
                            start=False, stop=True,
                        )
                ctx[m] = dict(scores=scores_ps)

            def stage_stats(m):
                cm = cmac[m]
                subs = subs_of(cm)
                ns = len(subs)
                scores_ps = ctx[m]["scores"]
                rmax = max(r for _, r in subs)
                req = all(r == rmax for _, r in subs)
                negmax = mpool.tile([P, 4], F32, tag="negmax")
                ebias = mpool.tile([P, 4], F32, tag="ebias")
                if req:
                    nc.vector.tensor_reduce(
                        negmax[:rmax, :ns],
                        scores_ps[:rmax, :ns * DF].rearrange(
                            "p (s j) -> p s j", s=ns),
                        axis=mybir.AxisListType.X,
                        op=mybir.AluOpType.max,
                        negate=True,
                    )
                    nc.gpsimd.tensor_scalar_mul(
                        ebias[:rmax, :ns], negmax[:rmax, :ns], ISC)
                else:
                    for s, (io, r) in enumerate(subs):
                        nc.vector.tensor_reduce(
                            negmax[:r, s:s + 1],
                            scores_ps[:r, s * DF:(s + 1) * DF],
                            axis=mybir.AxisListType.X,
                            op=mybir.AluOpType.max,
                            negate=True,
                        )
                        nc.gpsimd.tensor_scalar_mul(
                            ebias[:r, s:s + 1], negmax[:r, s:s + 1], ISC)
                attnU = mpool.tile([P, 4, DF], F16, tag="attnU")
                for s, (io, r) in enumerate(subs):
                    nc.scalar.activation(
                        attnU[:r, s, :],
                        scores_ps[:r, s * DF:(s + 1) * DF],
                        Exp,
                        bias=ebias[:r, s:s + 1],
                        scale=ISC,
                    )
                ctx[m]["attnU"] = attnU

            def stage_transp(m):
                cm = cmac[m]
                attnU = ctx[m]["attnU"]
                aT_ps = pp.tile([P, 2, CMACRO], F16, tag="attnT", bufs=2)
                for s, (io, r) in enumerate(subs_of(cm)):
                    rp = r + (r & 1)
                    for jh in range(2):
                        nc.tensor.transpose(
                            aT_ps[:, jh, io:io + rp],
                            attnU[:rp, s, jh * P:(jh + 1) * P],
                            eye_sb[:rp, :rp],
                        )
                ctx[m]["aT_ps"] = aT_ps

            def stage_attnT_sum(m):
                cm = cmac[m]
                subs = subs_of(cm)
                ns = len(subs)
                gm = cm["gm"]
                attnU = ctx[m]["attnU"]
                sum_sb = sums[cm["b"]]
                rmax = max(r for _, r in subs)
                req = all(r == rmax for _, r in subs)
                with nc.allow_low_precision("fp16 sumexp is plenty (<=2^-11 rel)"):
                    if req:
                        nc.vector.tensor_reduce(
                            sum_sb[:rmax, 4 * gm:4 * gm + ns],
                            attnU[:rmax, :ns, :],
                            axis=mybir.AxisListType.X,
                            op=mybir.AluOpType.add,
                        )
                    else:
                        for s, (io, r) in enumerate(subs):
                            nc.vector.tensor_reduce(
                                sum_sb[:r, 4 * gm + s:4 * gm + s + 1],
                                attnU[:r, s, :],
                                axis=mybir.AxisListType.X,
                                op=mybir.AluOpType.add,
                            )
                attnT = mpool.tile([P, 2, CMACRO], F16, tag="attnTsb")
                nc.vector.tensor_copy(attnT[:, :, :cm["R"]],
                                      ctx[m]["aT_ps"][:, :, :cm["R"]])
                ctx[m]["attnT"] = attnT
                if cm["last_in_batch"]:
                    nc.sync.dma_start(sum_d[cm["b"]], sum_sb[:])

            def stage_final(m):
                cm = cmac[m]
                R = cm["R"]
                b = cm["b"]
                attnT = ctx[m]["attnT"]
                if cm["first_in_chunk"]:
                    outT_sb = opool.tile([P, 2, DMA_MACRO], F16, tag="outT")
                    outbuf[cm["chunk"]] = outT_sb
                fin_ps = pp.tile([P, 2, CMACRO], F32, tag="fin", bufs=1)
                for c in range(2):
                    for k in range(2):
                        nc.tensor.matmul(
                            fin_ps[:, c, :R],
                            wo_sb[:, b, k, c * P:(c + 1) * P],
                            attnT[:, k, :R],
                            start=(k == 0), stop=(k == 1),
                        )
                ctx[m]["fin"] = fin_ps

            def stage_outT(m):
                cm = cmac[m]
                outT_sb = outbuf[cm["chunk"]]
                nc.scalar.activation(
                    outT_sb[:, :, cm["mo"]:cm["mo"] + cm["R"]],
                    ctx[m]["fin"][:, :, :cm["R"]], Copy)
                if cm["last_in_chunk"]:
                    b, d0, dlen = cm["b"], cm["d0"], cm["dlen"]
                    nc.sync.dma_start(
                        out_t[b][:, d0:d0 + dlen].rearrange(
                            "(c p) i -> p c i", p=P),
                        outT_sb[:, :, :dlen],
                    )
                ctx[m] = None

            # ---- software-pipelined emission ----
            for m in range(M + 2):
                if m < M:
                    stage_scores(m)
                if 2 <= m and m - 2 < M:
                    stage_final(m - 2)
                if 1 <= m and m - 1 < M:
                    stage_transp(m - 1)
                if 1 <= m and m - 1 < M:
                    stage_attnT_sum(m - 1)
                if m < M:
                    stage_stats(m)
                if 2 <= m and m - 2 < M:
                    stage_outT(m - 2)

    nc.compile()
    return nc


_PROGRAM_CACHE = {}


def _get_program(seq=SEQ, batches=BATCHES_PER_CORE, use_ba=False):
    key = (seq, batches, use_ba)
    if key not in _PROGRAM_CACHE:
        _PROGRAM_CACHE[key] = build_program(seq, batches, use_ba)
    return _PROGRAM_CACHE[key]


def make_in_maps(input_a, input_b, Wa, ba, Wb, bb, Wc, bc,
                 n_cores=N_CORES, batches=BATCHES_PER_CORE):
    input_a = np.asarray(input_a, dtype=np.float32)
    input_b = np.asarray(input_b, dtype=np.float32)
    Wa = np.asarray(Wa, np.float32)
    Wb = np.asarray(Wb, np.float32)
    Wc = np.asarray(Wc, np.float32)
    ba = np.asarray(ba, np.float32)
    bb = np.asarray(bb, np.float32)
    bc = np.asarray(bc, np.float32)

    a_t = np.ascontiguousarray(input_a.transpose(0, 2, 1))      # [B, DF, seq]
    B, _, seq = a_t.shape
    a16 = a_t.astype(np.float16)
    a16s = (a16.astype(np.float32) * 32.0).astype(np.float16)
    alo8 = ((a_t - a16.astype(np.float32)) * SC).astype(ml_dtypes.float8_e4m3)
    a8 = a_t.astype(ml_dtypes.float8_e4m3)
    a8pair = np.empty((B, P, 2, 2, seq), dtype=ml_dtypes.float8_e4m3)
    a8pair[:, :, :, 0, :] = alo8.reshape(B, 2, P, seq).transpose(0, 2, 1, 3)
    a8pair[:, :, :, 1, :] = a8.reshape(B, 2, P, seq).transpose(0, 2, 1, 3)
    a16v = np.ascontiguousarray(a16s.reshape(B, 2, P, seq))

    # host-side weight prep (exact fp32)
    mapped_b = np.einsum("bjf,fh->bjh", input_b, Wb) + bb       # [B, 256, 64]
    wf = 8.0 * np.einsum("fh,bjh->bfj", Wa, mapped_b)           # [B, 256(f), 256(j)]
    w16 = wf.astype(np.float16).astype(np.float32)
    w16s = (w16 * 64.0).astype(np.float16)                      # fp16(Wf)*64
    wlo8 = ((wf - w16) * SC).astype(ml_dtypes.float8_e4m3)
    w8 = wf.astype(ml_dtypes.float8_e4m3)
    # [B, 128(f%128), 2(fchunk), 2(w8|wlo8), 256(j)]
    w8pair = np.empty((B, P, 2, 2, DF), dtype=ml_dtypes.float8_e4m3)
    w8pair[:, :, :, 0, :] = w8.reshape(B, 2, P, DF).transpose(0, 2, 1, 3)
    w8pair[:, :, :, 1, :] = wlo8.reshape(B, 2, P, DF).transpose(0, 2, 1, 3)
    w16v = np.ascontiguousarray(
        w16s.reshape(B, 2, P, DF).transpose(0, 2, 1, 3))        # [B, 128, 2, 256]

    wout = np.einsum("bjh,hf->bjf", mapped_b, Wc) + bc          # [B, 256(j), 256(f)]
    wo16 = np.ascontiguousarray(
        wout.astype(np.float16).reshape(B, 2, P, DF).transpose(0, 2, 1, 3))
    sbias = (8.0 * SC) * np.einsum("h,bjh->bj", ba, mapped_b)   # [B, 256(j)]
    sbias = np.ascontiguousarray(sbias.reshape(B, 1, DF).astype(np.float32))

    shared = {
        "eye_d": np.eye(P, dtype=np.float16),
        "ones_d": np.ones((1, P), dtype=np.float32),
    }
    in_maps = []
    for c in range(n_cores):
        lo, hi = c * batches, (c + 1) * batches
        in_maps.append({
            "a16_d": np.ascontiguousarray(a16v[lo:hi]),
            "a8_d": np.ascontiguousarray(a8pair[lo:hi]),
            "w16_d": np.ascontiguousarray(w16v[lo:hi]),
            "w8_d": np.ascontiguousarray(w8pair[lo:hi]),
            "wo_d": np.ascontiguousarray(wo16[lo:hi]),
            "sb_d": np.ascontiguousarray(sbias[lo:hi]),
            **shared,
        })
    return in_maps


def _postprocess(results, seq=SEQ, batches=BATCHES_PER_CORE):
    """Concatenate per-core outputs, transpose, and normalize by sumexp."""
    outs = np.concatenate(
        [np.asarray(r["out_t"], dtype=np.float32) for r in results], axis=0)
    sums = np.concatenate(
        [np.asarray(r["sum_d"], dtype=np.float32) for r in results], axis=0)
    B = outs.shape[0]
    # sums[b, p, 4*gm + s] -> row i = 512*gm + 128*s + p
    n_cm = sums.shape[2] // 4
    grid = sums.reshape(B, P, n_cm, 4).transpose(0, 2, 3, 1).reshape(B, -1)
    se = grid[:, :seq]                                  # [B, seq]
    out = outs.transpose(0, 2, 1) / se[:, :, None]
    return np.ascontiguousarray(out.astype(np.float32))


def kernel(input_a, input_b, Wa, ba, Wb, bb, Wc, bc):
    use_ba = bool(np.any(np.asarray(ba)))
    nc = _get_program(use_ba=use_ba)
    in_maps = make_in_maps(input_a, input_b, Wa, ba, Wb, bb, Wc, bc)
    res = run_bass_kernel_spmd(nc, in_maps, core_ids=list(range(N_CORES)))
    return _postprocess(res.results)


# revision 18
# speedup vs baseline: 1.5334x; 1.2324x over previous
"""Trainium2 Bass kernel for nn_CrossAttention (16x6209x256 cross-attention).

Strategy
--------
Data-parallel over batch: 16 batches -> 8 cores x 2 batches, pure SPMD.

All weight prep happens on the HOST (tiny matmuls, exact fp32):
    mapped_b = b @ Wb + bb                        [256, 64]
    Wf       = 8 * Wa @ mapped_b.T                [256, 256]
    Wout     = mapped_b @ Wc + 1 x bc             [256, 256]
The device computes, per batch:
    scores = a @ Wf  (at a 2^11 PSUM scale) as
        fp16(a)*32 @ fp16(Wf)*64                  (fp16 hi term, 2 matmuls)
      + e4m3(alo*2^11) @ e4m3(Wf)                 } one fp8 DoubleRow pair
      + e4m3(a)        @ e4m3(Wlo*2^11)           } per k-chunk
    attnU  = exp(scores*2^-11 - max)              fp16 (unnormalized)
    sumexp = sum_j attnU                          (DVE reduce, fp16)
    outT   = Wout^T @ attnU^T                     (PE transpose + fp16 matmul)
Host postprocess: out = outT.T / sumexp (+exact bc since sum(attnU)=sumexp).
rel err ~1.7e-3 (validated in simulation against the fp32 reference).

The main loop is software-pipelined: per iteration m the PE stream is
scores(m), final(m-2), transp(m-1) so every cross-engine dependency has a
full iteration of slack; DVE runs sumexp(m-1), attnT copy(m-1), reduce(m);
ACT runs exp(m) then outT(m-2).
"""
import sys

for _p in ("/opt/trn_rl_repo",):
    if _p not in sys.path:
        sys.path.append(_p)

import numpy as np
import ml_dtypes

import concourse.bacc as bacc
import concourse.mybir as mybir
import concourse.tile as tile
from concourse.bass_utils import run_bass_kernel_spmd

F32 = mybir.dt.float32
F16 = mybir.dt.float16
F8 = mybir.dt.float8e4
P = 128

N_CORES = 8
BATCHES_PER_CORE = 2
SEQ = 6209
DF = 256          # feature dim of a / b
HID = 64          # projection dim
DMA_MACRO = 2048  # rows fetched/stored per DMA instruction
CMACRO = 512      # rows per compute macro (4 subtiles of 128)

SC = 2048.0       # 2^11 PSUM score scale
ISC = 1.0 / SC


def _row_plan(n_rows):
    """[(dma_start, dma_len, [(cm_start_within_dma, cm_len), ...]), ...]"""
    plan = []
    pos = 0
    while pos < n_rows:
        d = min(DMA_MACRO, n_rows - pos)
        cms = []
        q = 0
        while q < d:
            c = min(CMACRO, d - q)
            cms.append((q, c))
            q += c
        plan.append((pos, d, cms))
        pos += d
    return plan


def _cmacro_list(seq, batches):
    out = []
    for b in range(batches):
        plan = _row_plan(seq)
        gm = 0
        for ci, (d0, dlen, cms) in enumerate(plan):
            for mi, (mo, R) in enumerate(cms):
                out.append(dict(
                    b=b, d0=d0, dlen=dlen, mo=mo, R=R,
                    chunk=(b, ci),
                    first_in_chunk=(mi == 0), last_in_chunk=(mi == len(cms) - 1),
                    last_in_batch=(ci == len(plan) - 1 and mi == len(cms) - 1),
                    gm=gm,
                ))
                gm += 1
    return out


def build_program(seq=SEQ, batches=BATCHES_PER_CORE, use_ba=False):
    nc = bacc.Bacc("TRN2", target_bir_lowering=False, debug=False)

    a16_d = nc.dram_tensor("a16_d", [batches, 2, P, seq], F16, kind="ExternalInput")
    a8_d = nc.dram_tensor("a8_d", [batches, P, 2, 2, seq], F8, kind="ExternalInput")
    w16_d = nc.dram_tensor("w16_d", [batches, P, 2, DF], F16, kind="ExternalInput")
    w8_d = nc.dram_tensor("w8_d", [batches, P, 2, 2, DF], F8, kind="ExternalInput")
    wo_d = nc.dram_tensor("wo_d", [batches, P, 2, DF], F16, kind="ExternalInput")
    sb_d = nc.dram_tensor("sb_d", [batches, 1, DF], F32, kind="ExternalInput")
    eye_d = nc.dram_tensor("eye_d", [P, P], F16, kind="ExternalInput")
    ones_d = nc.dram_tensor("ones_d", [1, P], F32, kind="ExternalInput")
    out_t = nc.dram_tensor("out_t", [batches, DF, seq], F16, kind="ExternalOutput")
    n_sumcol = 4 * len([c for _, _, cs in _row_plan(seq) for c in cs])
    sum_d = nc.dram_tensor("sum_d", [batches, P, n_sumcol], F16,
                           kind="ExternalOutput")

    Exp = mybir.ActivationFunctionType.Exp
    Copy = mybir.ActivationFunctionType.Copy
    DR = mybir.MatmulPerfMode.DoubleRow

    with tile.TileContext(nc) as tc:
        with (
            tc.tile_pool(name="const", bufs=1) as cpool,
            tc.tile_pool(name="apool", bufs=3) as apool,
            tc.tile_pool(name="mpool", bufs=2) as mpool,
            tc.tile_pool(name="opool", bufs=3) as opool,
            tc.tile_pool(name="pp", bufs=1, space="PSUM") as pp,
        ):
            # ---- constants and per-batch weights (host-prepped) ----
            eye_sb = cpool.tile([P, P], F16)
            nc.sync.dma_start(eye_sb[:], eye_d[:])
            ones_sb = cpool.tile([1, P], F32)
            nc.sync.dma_start(ones_sb[:], ones_d[:])
            w16_sb = cpool.tile([P, batches, 2, DF], F16)
            nc.sync.dma_start(w16_sb[:], w16_d[:].rearrange("b p k j -> p b k j"))
            w8_sb = cpool.tile([P, batches, 2, 2, DF], F8)
            nc.sync.dma_start(w8_sb[:], w8_d[:].rearrange("b p k t j -> p b k t j"))
            wo_sb = cpool.tile([P, batches, 2, DF], F16)
            nc.sync.dma_start(wo_sb[:], wo_d[:].rearrange("b p k f -> p b k f"))
            sbias_sb = cpool.tile([1, batches, DF], F32)
            nc.sync.dma_start(sbias_sb[:], sb_d[:].rearrange("b x j -> x b j"))

            cmac = _cmacro_list(seq, batches)
            M = len(cmac)
            sums = [None] * batches
            ain = {}
            outbuf = {}
            ctx = [None] * M

            def fetch_chunk(cm):
                key = cm["chunk"]
                if key in ain:
                    return
                b, d0, dlen = cm["b"], cm["d0"], cm["dlen"]
                a16_sb = apool.tile([P, 2, DMA_MACRO], F16, tag="a16")
                nc.sync.dma_start(
                    a16_sb[:, :, :dlen],
                    a16_d[b][:, :, d0:d0 + dlen].rearrange("k p i -> p k i"),
                )
                a8_sb = apool.tile([P, 2, 2, DMA_MACRO], F8, tag="a8")
                nc.sync.dma_start(
                    a8_sb[:, :, :, :dlen],
                    a8_d[b][:, :, :, d0:d0 + dlen],
                )
                ain[key] = (a16_sb, a8_sb)

            def subs_of(cm):
                return [(o, min(P, cm["R"] - o)) for o in range(0, cm["R"], P)]

            def stage_scores(m):
                cm = cmac[m]
                b = cm["b"]
                if cm["gm"] == 0 and sums[b] is None:
                    sum_sb = mpool.tile([P, n_sumcol], F16, tag=f"sums{b}", bufs=1)
                    sums[b] = sum_sb
                fetch_chunk(cm)
                nxt = next((c for c in cmac[m + 1:] if c["chunk"] != cm["chunk"]),
                           None)
                if nxt is not None:
                    fetch_chunk(nxt)
                a16_sb, a8_sb = ain[cm["chunk"]]
                scores_ps = pp.tile([P, 4 * DF], F32, tag="scores", bufs=2)
                for s, (io, r) in enumerate(subs_of(cm)):
                    c0 = s * DF
                    go = cm["mo"] + io
                    for k in range(2):
                        nc.tensor.matmul(
                            scores_ps[:r, c0:c0 + DF],
                            a16_sb[:, k, go:go + r],
                            w16_sb[:, b, k, :],
                            start=(k == 0), stop=False,
                        )
                    for k in range(2):
                        nc.tensor.matmul(
                            scores_ps[:r, c0:c0 + DF],
                            a8_sb[:, k, :, go:go + r],
                            w8_sb[:, b, k, :, :],
                            start=False,
                            stop=(k == 1) and not use_ba,
                            perf_mode=DR,
                        )
                    if use_ba:
                        nc.tensor.matmul(
                            scores_ps[:r, c0:c0 + DF],
                            ones_sb[:, :r],
                            sbias_sb[:, b, :],
                            start=False, stop=True,
                        )
                ctx[m] = dict(scores=scores_ps)

            def stage_stats(m):
                cm = cmac[m]
                subs = subs_of(cm)
                ns = len(subs)
                scores_ps = ctx[m]["scores"]
                rmax = max(r for _, r in subs)
                req = all(r == rmax for _, r in subs)
                negmax = mpool.tile([P, 4], F32, tag="negmax")
                ebias = mpool.tile([P, 4], F32, tag="ebias")
                if req:
                    nc.vector.tensor_reduce(
                        negmax[:rmax, :ns],
                        scores_ps[:rmax, :ns * DF].rearrange(
                            "p (s j) -> p s j", s=ns),
                        axis=mybir.AxisListType.X,
                        op=mybir.AluOpType.max,
                        negate=True,
                    )
                    nc.vector.tensor_scalar_mul(
                        ebias[:rmax, :ns], negmax[:rmax, :ns], ISC)
                else:
                    for s, (io, r) in enumerate(subs):
                        nc.vector.tensor_reduce(
                            negmax[:r, s:s + 1],
                            scores_ps[:r, s * DF:(s + 1) * DF],
                            axis=mybir.AxisListType.X,
                            op=mybir.AluOpType.max,
                            negate=True,
                        )
                        nc.vector.tensor_scalar_mul(
                            ebias[:r, s:s + 1], negmax[:r, s:s + 1], ISC)
                attnU = mpool.tile([P, 4, DF], F16, tag="attnU", bufs=3)
                for s, (io, r) in enumerate(subs):
                    nc.scalar.activation(
                        attnU[:r, s, :],
                        scores_ps[:r, s * DF:(s + 1) * DF],
                        Exp,
                        bias=ebias[:r, s:s + 1],
                        scale=ISC,
                    )
                ctx[m]["attnU"] = attnU

            def stage_transp(m):
                cm = cmac[m]
                attnU = ctx[m]["attnU"]
                aT_ps = pp.tile([P, 2, CMACRO], F16, tag="attnT", bufs=2)
                for s, (io, r) in enumerate(subs_of(cm)):
                    rp = r + (r & 1)
                    for jh in range(2):
                        nc.tensor.transpose(
                            aT_ps[:, jh, io:io + rp],
                            attnU[:rp, s, jh * P:(jh + 1) * P],
                            eye_sb[:rp, :rp],
                        )
                ctx[m]["aT_ps"] = aT_ps

            def stage_attnT_sum(m):
                cm = cmac[m]
                subs = subs_of(cm)
                ns = len(subs)
                gm = cm["gm"]
                attnU = ctx[m]["attnU"]
                sum_sb = sums[cm["b"]]
                rmax = max(r for _, r in subs)
                req = all(r == rmax for _, r in subs)
                with nc.allow_low_precision("fp16 sumexp is plenty (<=2^-11 rel)"):
                    if req:
                        nc.vector.tensor_reduce(
                            sum_sb[:rmax, 4 * gm:4 * gm + ns],
                            attnU[:rmax, :ns, :],
                            axis=mybir.AxisListType.X,
                            op=mybir.AluOpType.add,
                        )
                    else:
                        for s, (io, r) in enumerate(subs):
                            nc.vector.tensor_reduce(
                                sum_sb[:r, 4 * gm + s:4 * gm + s + 1],
                                attnU[:r, s, :],
                                axis=mybir.AxisListType.X,
                                op=mybir.AluOpType.add,
                            )
                attnT = mpool.tile([P, 2, CMACRO], F16, tag="attnTsb")
                nc.vector.tensor_copy(attnT[:, :, :cm["R"]],
                                      ctx[m]["aT_ps"][:, :, :cm["R"]])
                ctx[m]["attnT"] = attnT
                if cm["last_in_batch"]:
                    nc.sync.dma_start(sum_d[cm["b"]], sum_sb[:])

            def stage_final(m):
                cm = cmac[m]
                R = cm["R"]
                b = cm["b"]
                attnT = ctx[m]["attnT"]
                if cm["first_in_chunk"]:
                    outT_sb = opool.tile([P, 2, DMA_MACRO], F16, tag="outT")
                    outbuf[cm["chunk"]] = outT_sb
                fin_ps = pp.tile([P, 2, CMACRO], F32, tag="fin", bufs=1)
                for c in range(2):
                    for k in range(2):
                        nc.tensor.matmul(
                            fin_ps[:, c, :R],
                            wo_sb[:, b, k, c * P:(c + 1) * P],
                            attnT[:, k, :R],
                            start=(k == 0), stop=(k == 1),
                        )
                ctx[m]["fin"] = fin_ps

            def stage_outT(m):
                cm = cmac[m]
                outT_sb = outbuf[cm["chunk"]]
                nc.scalar.activation(
                    outT_sb[:, :, cm["mo"]:cm["mo"] + cm["R"]],
                    ctx[m]["fin"][:, :, :cm["R"]], Copy)
                if cm["last_in_chunk"]:
                    b, d0, dlen = cm["b"], cm["d0"], cm["dlen"]
                    nc.sync.dma_start(
                        out_t[b][:, d0:d0 + dlen].rearrange(
                            "(c p) i -> p c i", p=P),
                        outT_sb[:, :, :dlen],
                    )
                ctx[m] = None

            # ---- software-pipelined emission ----
            # PE: scores(m), final(m-3), transp(m-2)
            # DVE: reduce+ebias(m), sumexp(m-2), attnT copy(m-2)
            # ACT: exp(m), outT(m-3)
            for m in range(M + 3):
                if m < M:
                    stage_scores(m)
                if 3 <= m and m - 3 < M:
                    stage_final(m - 3)
                if 2 <= m and m - 2 < M:
                    stage_transp(m - 2)
                if m < M:
                    stage_stats(m)
                if 2 <= m and m - 2 < M:
                    stage_attnT_sum(m - 2)
                if 3 <= m and m - 3 < M:
                    stage_outT(m - 3)

    nc.compile()
    return nc


_PROGRAM_CACHE = {}


def _get_program(seq=SEQ, batches=BATCHES_PER_CORE, use_ba=False):
    key = (seq, batches, use_ba)
    if key not in _PROGRAM_CACHE:
        _PROGRAM_CACHE[key] = build_program(seq, batches, use_ba)
    return _PROGRAM_CACHE[key]


def make_in_maps(input_a, input_b, Wa, ba, Wb, bb, Wc, bc,
                 n_cores=N_CORES, batches=BATCHES_PER_CORE):
    input_a = np.asarray(input_a, dtype=np.float32)
    input_b = np.asarray(input_b, dtype=np.float32)
    Wa = np.asarray(Wa, np.float32)
    Wb = np.asarray(Wb, np.float32)
    Wc = np.asarray(Wc, np.float32)
    ba = np.asarray(ba, np.float32)
    bb = np.asarray(bb, np.float32)
    bc = np.asarray(bc, np.float32)

    a_t = np.ascontiguousarray(input_a.transpose(0, 2, 1))      # [B, DF, seq]
    B, _, seq = a_t.shape
    a16 = a_t.astype(np.float16)
    a16s = (a16.astype(np.float32) * 32.0).astype(np.float16)
    alo8 = ((a_t - a16.astype(np.float32)) * SC).astype(ml_dtypes.float8_e4m3)
    a8 = a_t.astype(ml_dtypes.float8_e4m3)
    a8pair = np.empty((B, P, 2, 2, seq), dtype=ml_dtypes.float8_e4m3)
    a8pair[:, :, :, 0, :] = alo8.reshape(B, 2, P, seq).transpose(0, 2, 1, 3)
    a8pair[:, :, :, 1, :] = a8.reshape(B, 2, P, seq).transpose(0, 2, 1, 3)
    a16v = np.ascontiguousarray(a16s.reshape(B, 2, P, seq))

    # host-side weight prep (exact fp32)
    mapped_b = np.einsum("bjf,fh->bjh", input_b, Wb) + bb       # [B, 256, 64]
    wf = 8.0 * np.einsum("fh,bjh->bfj", Wa, mapped_b)           # [B, 256(f), 256(j)]
    w16 = wf.astype(np.float16).astype(np.float32)
    w16s = (w16 * 64.0).astype(np.float16)                      # fp16(Wf)*64
    wlo8 = ((wf - w16) * SC).astype(ml_dtypes.float8_e4m3)
    w8 = wf.astype(ml_dtypes.float8_e4m3)
    # [B, 128(f%128), 2(fchunk), 2(w8|wlo8), 256(j)]
    w8pair = np.empty((B, P, 2, 2, DF), dtype=ml_dtypes.float8_e4m3)
    w8pair[:, :, :, 0, :] = w8.reshape(B, 2, P, DF).transpose(0, 2, 1, 3)
    w8pair[:, :, :, 1, :] = wlo8.reshape(B, 2, P, DF).transpose(0, 2, 1, 3)
    w16v = np.ascontiguousarray(
        w16s.reshape(B, 2, P, DF).transpose(0, 2, 1, 3))        # [B, 128, 2, 256]

    wout = np.einsum("bjh,hf->bjf", mapped_b, Wc) + bc          # [B, 256(j), 256(f)]
    wo16 = np.ascontiguousarray(
        wout.astype(np.float16).reshape(B, 2, P, DF).transpose(0, 2, 1, 3))
    sbias = (8.0 * SC) * np.einsum("h,bjh->bj", ba, mapped_b)   # [B, 256(j)]
    sbias = np.ascontiguousarray(sbias.reshape(B, 1, DF).astype(np.float32))

    shared = {
        "eye_d": np.eye(P, dtype=np.float16),
        "ones_d": np.ones((1, P), dtype=np.float32),
    }
    in_maps = []
    for c in range(n_cores):
        lo, hi = c * batches, (c + 1) * batches
        in_maps.append({
            "a16_d": np.ascontiguousarray(a16v[lo:hi]),
            "a8_d": np.ascontiguousarray(a8pair[lo:hi]),
            "w16_d": np.ascontiguousarray(w16v[lo:hi]),
            "w8_d": np.ascontiguousarray(w8pair[lo:hi]),
            "wo_d": np.ascontiguousarray(wo16[lo:hi]),
            "sb_d": np.ascontiguousarray(sbias[lo:hi]),
            **shared,
        })
    return in_maps


def _postprocess(results, seq=SEQ, batches=BATCHES_PER_CORE):
    """Concatenate per-core outputs, transpose, and normalize by sumexp."""
    outs = np.concatenate(
        [np.asarray(r["out_t"], dtype=np.float32) for r in results], axis=0)
    sums = np.concatenate(
        [np.asarray(r["sum_d"], dtype=np.float32) for r in results], axis=0)
    B = outs.shape[0]
    # sums[b, p, 4*gm + s] -> row i = 512*gm + 128*s + p
    n_cm = sums.shape[2] // 4
    grid = sums.reshape(B, P, n_cm, 4).transpose(0, 2, 3, 1).reshape(B, -1)
    se = grid[:, :seq]                                  # [B, seq]
    out = outs.transpose(0, 2, 1) / se[:, :, None]
    return np.ascontiguousarray(out.astype(np.float32))


def kernel(input_a, input_b, Wa, ba, Wb, bb, Wc, bc):
    use_ba = bool(np.any(np.asarray(ba)))
    nc = _get_program(use_ba=use_ba)
    in_maps = make_in_maps(input_a, input_b, Wa, ba, Wb, bb, Wc, bc)
    res = run_bass_kernel_spmd(nc, in_maps, core_ids=list(range(N_CORES)))
    return _postprocess(res.results)


# revision 21
# speedup vs baseline: 1.7867x; 1.1652x over previous
"""Trainium2 Bass kernel for nn_CrossAttention (16x6209x256 cross-attention).

Strategy
--------
Data-parallel over batch: 16 batches -> 8 cores x 2 batches, pure SPMD.

All weight prep happens on the HOST (tiny matmuls, exact fp32):
    mapped_b = b @ Wb + bb                        [256, 64]
    Wf       = 8 * Wa @ mapped_b.T                [256, 256]
    Wout     = mapped_b @ Wc + 1 x bc             [256, 256]
The device computes, per batch:
    scores = a @ Wf  (at a 2^11 PSUM scale) as
        fp16(a)*32 @ fp16(Wf)*64                  (fp16 hi term, 2 matmuls)
      + e4m3(alo*2^11) @ e4m3(Wf)                 } one fp8 DoubleRow pair
      + e4m3(a)        @ e4m3(Wlo*2^11)           } per k-chunk
    attnU  = exp(scores*2^-11 - max)              fp16 (unnormalized)
    sumexp = sum_j attnU                          (DVE reduce, fp16)
    outT   = Wout^T @ attnU^T                     (PE transpose + fp16 matmul)
Host postprocess: out = outT.T / sumexp (+exact bc since sum(attnU)=sumexp).
rel err ~1.7e-3 (validated in simulation against the fp32 reference).

The main loop is software-pipelined: per iteration m the PE stream is
scores(m), final(m-2), transp(m-1) so every cross-engine dependency has a
full iteration of slack; DVE runs sumexp(m-1), attnT copy(m-1), reduce(m);
ACT runs exp(m) then outT(m-2).
"""
import sys

for _p in ("/opt/trn_rl_repo",):
    if _p not in sys.path:
        sys.path.append(_p)

import numpy as np
import ml_dtypes

import concourse.bacc as bacc
import concourse.mybir as mybir
import concourse.tile as tile
from concourse.bass_utils import run_bass_kernel_spmd

F32 = mybir.dt.float32
F16 = mybir.dt.float16
F8 = mybir.dt.float8e4
P = 128

N_CORES = 8
BATCHES_PER_CORE = 2
SEQ = 6209
DF = 256          # feature dim of a / b
HID = 64          # projection dim
DMA_MACRO = 2048  # rows fetched/stored per DMA instruction
CMACRO = 512      # rows per compute macro (4 subtiles of 128)

SC = 2048.0       # 2^11 PSUM score scale
ISC = 1.0 / SC


def _row_plan(n_rows):
    """[(dma_start, dma_len, [(cm_start_within_dma, cm_len), ...]), ...]"""
    plan = []
    pos = 0
    while pos < n_rows:
        d = min(DMA_MACRO, n_rows - pos)
        cms = []
        q = 0
        while q < d:
            c = min(CMACRO, d - q)
            cms.append((q, c))
            q += c
        plan.append((pos, d, cms))
        pos += d
    return plan


def _cmacro_list(seq, batches):
    out = []
    for b in range(batches):
        plan = _row_plan(seq)
        gm = 0
        for ci, (d0, dlen, cms) in enumerate(plan):
            for mi, (mo, R) in enumerate(cms):
                out.append(dict(
                    b=b, d0=d0, dlen=dlen, mo=mo, R=R,
                    chunk=(b, ci),
                    first_in_chunk=(mi == 0), last_in_chunk=(mi == len(cms) - 1),
                    last_in_batch=(ci == len(plan) - 1 and mi == len(cms) - 1),
                    gm=gm,
                ))
                gm += 1
    return out


def build_program(seq=SEQ, batches=BATCHES_PER_CORE, use_ba=False):
    nc = bacc.Bacc("TRN2", target_bir_lowering=False, debug=False)

    a16_d = nc.dram_tensor("a16_d", [batches, 2, P, seq], F16, kind="ExternalInput")
    a8_d = nc.dram_tensor("a8_d", [batches, P, 2, 2, seq], F8, kind="ExternalInput")
    w16_d = nc.dram_tensor("w16_d", [batches, P, 2, DF], F16, kind="ExternalInput")
    w8_d = nc.dram_tensor("w8_d", [batches, P, 2, 2, DF], F8, kind="ExternalInput")
    wo_d = nc.dram_tensor("wo_d", [batches, P, 2, DF], F16, kind="ExternalInput")
    sb_d = nc.dram_tensor("sb_d", [batches, 1, DF], F32, kind="ExternalInput")
    eye_d = nc.dram_tensor("eye_d", [P, P], F16, kind="ExternalInput")
    ones_d = nc.dram_tensor("ones_d", [1, P], F32, kind="ExternalInput")
    out_t = nc.dram_tensor("out_t", [batches, DF, seq], F16, kind="ExternalOutput")
    n_sumcol = 4 * len([c for _, _, cs in _row_plan(seq) for c in cs])
    sum_d = nc.dram_tensor("sum_d", [batches, P, n_sumcol], F16,
                           kind="ExternalOutput")

    Exp = mybir.ActivationFunctionType.Exp
    Copy = mybir.ActivationFunctionType.Copy
    DR = mybir.MatmulPerfMode.DoubleRow

    with tile.TileContext(nc) as tc:
        with (
            tc.tile_pool(name="const", bufs=1) as cpool,
            tc.tile_pool(name="apool", bufs=3) as apool,
            tc.tile_pool(name="mpool", bufs=2) as mpool,
            tc.tile_pool(name="opool", bufs=3) as opool,
            tc.tile_pool(name="pp", bufs=1, space="PSUM") as pp,
        ):
            # ---- constants and per-batch weights (host-prepped) ----
            eye_sb = cpool.tile([P, P], F16)
            nc.sync.dma_start(eye_sb[:], eye_d[:])
            ones_sb = cpool.tile([1, P], F32)
            nc.sync.dma_start(ones_sb[:], ones_d[:])
            w16_sb = cpool.tile([P, batches, 2, DF], F16)
            nc.sync.dma_start(w16_sb[:], w16_d[:].rearrange("b p k j -> p b k j"))
            w8_sb = cpool.tile([P, batches, 2, 2, DF], F8)
            nc.sync.dma_start(w8_sb[:], w8_d[:].rearrange("b p k t j -> p b k t j"))
            wo_sb = cpool.tile([P, batches, 2, DF], F16)
            nc.sync.dma_start(wo_sb[:], wo_d[:].rearrange("b p k f -> p b k f"))
            sbias_sb = cpool.tile([1, batches, DF], F32)
            nc.sync.dma_start(sbias_sb[:], sb_d[:].rearrange("b x j -> x b j"))

            cmac = _cmacro_list(seq, batches)
            M = len(cmac)
            sums = [None] * batches
            ain = {}
            outbuf = {}
            ctx = [None] * M

            def fetch_chunk(cm):
                key = cm["chunk"]
                if key in ain:
                    return
                b, d0, dlen = cm["b"], cm["d0"], cm["dlen"]
                a16_sb = apool.tile([P, 2, DMA_MACRO], F16, tag="a16")
                a8_sb = apool.tile([P, 2, 2, DMA_MACRO], F8, tag="a8")
                # First chunk of the program: land the first cmacro's rows
                # quickly so the pipeline starts early.
                pieces = [(0, CMACRO), (CMACRO, dlen - CMACRO)] \
                    if (key == (0, 0) and dlen > CMACRO) else [(0, dlen)]
                for o, ln in pieces:
                    nc.sync.dma_start(
                        a16_sb[:, :, o:o + ln],
                        a16_d[b][:, :, d0 + o:d0 + o + ln].rearrange(
                            "k p i -> p k i"),
                    )
                    nc.sync.dma_start(
                        a8_sb[:, :, :, o:o + ln],
                        a8_d[b][:, :, :, d0 + o:d0 + o + ln],
                    )
                ain[key] = (a16_sb, a8_sb)

            def subs_of(cm):
                return [(o, min(P, cm["R"] - o)) for o in range(0, cm["R"], P)]

            def stage_scores(m):
                cm = cmac[m]
                b = cm["b"]
                if cm["gm"] == 0 and sums[b] is None:
                    sum_sb = mpool.tile([P, n_sumcol], F16, tag=f"sums{b}", bufs=1)
                    sums[b] = sum_sb
                fetch_chunk(cm)
                nxt = next((c for c in cmac[m + 1:] if c["chunk"] != cm["chunk"]),
                           None)
                if nxt is not None:
                    fetch_chunk(nxt)
                a16_sb, a8_sb = ain[cm["chunk"]]
                subs = subs_of(cm)
                halves = []
                for h, hsubs in ((0, subs[:2]), (1, subs[2:])):
                    if not hsubs:
                        continue
                    sc_ps = pp.tile([P, 2 * DF], F32, tag=f"scores{h}", bufs=2)
                    halves.append(sc_ps)
                    for s, (io, r) in enumerate(hsubs):
                        c0 = s * DF
                        go = cm["mo"] + io
                        for k in range(2):
                            nc.tensor.matmul(
                                sc_ps[:r, c0:c0 + DF],
                                a16_sb[:, k, go:go + r],
                                w16_sb[:, b, k, :],
                                start=(k == 0), stop=False,
                            )
                        for k in range(2):
                            nc.tensor.matmul(
                                sc_ps[:r, c0:c0 + DF],
                                a8_sb[:, k, :, go:go + r],
                                w8_sb[:, b, k, :, :],
                                start=False,
                                stop=(k == 1) and not use_ba,
                                perf_mode=DR,
                            )
                        if use_ba:
                            nc.tensor.matmul(
                                sc_ps[:r, c0:c0 + DF],
                                ones_sb[:, :r],
                                sbias_sb[:, b, :],
                                start=False, stop=True,
                            )
                ctx[m] = dict(scores=halves)

            def stage_stats(m):
                cm = cmac[m]
                subs = subs_of(cm)
                halves = ctx[m]["scores"]
                negmax = mpool.tile([P, 4], F32, tag="negmax")
                ebias = mpool.tile([P, 4], F32, tag="ebias")
                attnU = mpool.tile([P, 4, DF], F16, tag="attnU", bufs=3)
                for h, sc_ps in enumerate(halves):
                    hsubs = subs[2 * h:2 * h + 2]
                    hn = len(hsubs)
                    rmax = max(r for _, r in hsubs)
                    req = all(r == rmax for _, r in hsubs)
                    hb = 2 * h
                    if req:
                        nc.vector.tensor_reduce(
                            negmax[:rmax, hb:hb + hn],
                            sc_ps[:rmax, :hn * DF].rearrange(
                                "p (s j) -> p s j", s=hn),
                            axis=mybir.AxisListType.X,
                            op=mybir.AluOpType.max,
                            negate=True,
                        )
                        nc.vector.tensor_scalar_mul(
                            ebias[:rmax, hb:hb + hn], negmax[:rmax, hb:hb + hn],
                            ISC)
                    else:
                        for s, (io, r) in enumerate(hsubs):
                            nc.vector.tensor_reduce(
                                negmax[:r, hb + s:hb + s + 1],
                                sc_ps[:r, s * DF:(s + 1) * DF],
                                axis=mybir.AxisListType.X,
                                op=mybir.AluOpType.max,
                                negate=True,
                            )
                            nc.vector.tensor_scalar_mul(
                                ebias[:r, hb + s:hb + s + 1],
                                negmax[:r, hb + s:hb + s + 1], ISC)
                    for s, (io, r) in enumerate(hsubs):
                        nc.scalar.activation(
                            attnU[:r, hb + s, :],
                            sc_ps[:r, s * DF:(s + 1) * DF],
                            Exp,
                            bias=ebias[:r, hb + s:hb + s + 1],
                            scale=ISC,
                        )
                ctx[m]["attnU"] = attnU

            def stage_transp(m):
                cm = cmac[m]
                attnU = ctx[m]["attnU"]
                aT_ps = pp.tile([P, 2, CMACRO], F16, tag="attnT", bufs=2)
                for s, (io, r) in enumerate(subs_of(cm)):
                    rp = r + (r & 1)
                    for jh in range(2):
                        nc.tensor.transpose(
                            aT_ps[:, jh, io:io + rp],
                            attnU[:rp, s, jh * P:(jh + 1) * P],
                            eye_sb[:rp, :rp],
                        )
                ctx[m]["aT_ps"] = aT_ps

            def stage_attnT_sum(m):
                cm = cmac[m]
                subs = subs_of(cm)
                ns = len(subs)
                gm = cm["gm"]
                attnU = ctx[m]["attnU"]
                sum_sb = sums[cm["b"]]
                rmax = max(r for _, r in subs)
                req = all(r == rmax for _, r in subs)
                with nc.allow_low_precision("fp16 sumexp is plenty (<=2^-11 rel)"):
                    if req:
                        nc.vector.tensor_reduce(
                            sum_sb[:rmax, 4 * gm:4 * gm + ns],
                            attnU[:rmax, :ns, :],
                            axis=mybir.AxisListType.X,
                            op=mybir.AluOpType.add,
                        )
                    else:
                        for s, (io, r) in enumerate(subs):
                            nc.vector.tensor_reduce(
                                sum_sb[:r, 4 * gm + s:4 * gm + s + 1],
                                attnU[:r, s, :],
                                axis=mybir.AxisListType.X,
                                op=mybir.AluOpType.add,
                            )
                attnT = mpool.tile([P, 2, CMACRO], F16, tag="attnTsb")
                nc.vector.tensor_copy(attnT[:, :, :cm["R"]],
                                      ctx[m]["aT_ps"][:, :, :cm["R"]])
                ctx[m]["attnT"] = attnT
                if cm["last_in_batch"]:
                    nc.sync.dma_start(sum_d[cm["b"]], sum_sb[:])

            def stage_final(m):
                cm = cmac[m]
                R = cm["R"]
                b = cm["b"]
                attnT = ctx[m]["attnT"]
                if cm["first_in_chunk"]:
                    outT_sb = opool.tile([P, 2, DMA_MACRO], F16, tag="outT")
                    outbuf[cm["chunk"]] = outT_sb
                fin_ps = pp.tile([P, 2, CMACRO], F32, tag="fin", bufs=1)
                for c in range(2):
                    for k in range(2):
                        nc.tensor.matmul(
                            fin_ps[:, c, :R],
                            wo_sb[:, b, k, c * P:(c + 1) * P],
                            attnT[:, k, :R],
                            start=(k == 0), stop=(k == 1),
                        )
                ctx[m]["fin"] = fin_ps

            def stage_outT(m):
                cm = cmac[m]
                outT_sb = outbuf[cm["chunk"]]
                nc.scalar.activation(
                    outT_sb[:, :, cm["mo"]:cm["mo"] + cm["R"]],
                    ctx[m]["fin"][:, :, :cm["R"]], Copy)
                if cm["last_in_chunk"]:
                    b, d0, dlen = cm["b"], cm["d0"], cm["dlen"]
                    nc.sync.dma_start(
                        out_t[b][:, d0:d0 + dlen].rearrange(
                            "(c p) i -> p c i", p=P),
                        outT_sb[:, :, :dlen],
                    )
                ctx[m] = None

            # ---- software-pipelined emission ----
            # PE: scores(m), final(m-3), transp(m-2)
            # DVE: reduce+ebias(m), sumexp(m-2), attnT copy(m-2)
            # ACT: exp(m), outT(m-3)
            for m in range(M + 3):
                if m < M:
                    stage_scores(m)
                if 3 <= m and m - 3 < M:
                    stage_final(m - 3)
                if 2 <= m and m - 2 < M:
                    stage_transp(m - 2)
                if m < M:
                    stage_stats(m)
                if 2 <= m and m - 2 < M:
                    stage_attnT_sum(m - 2)
                if 3 <= m and m - 3 < M:
                    stage_outT(m - 3)

    nc.compile()
    return nc


_PROGRAM_CACHE = {}


def _get_program(seq=SEQ, batches=BATCHES_PER_CORE, use_ba=False):
    key = (seq, batches, use_ba)
    if key not in _PROGRAM_CACHE:
        _PROGRAM_CACHE[key] = build_program(seq, batches, use_ba)
    return _PROGRAM_CACHE[key]


def make_in_maps(input_a, input_b, Wa, ba, Wb, bb, Wc, bc,
                 n_cores=N_CORES, batches=BATCHES_PER_CORE):
    input_a = np.asarray(input_a, dtype=np.float32)
    input_b = np.asarray(input_b, dtype=np.float32)
    Wa = np.asarray(Wa, np.float32)
    Wb = np.asarray(Wb, np.float32)
    Wc = np.asarray(Wc, np.float32)
    ba = np.asarray(ba, np.float32)
    bb = np.asarray(bb, np.float32)
    bc = np.asarray(bc, np.float32)

    a_t = np.ascontiguousarray(input_a.transpose(0, 2, 1))      # [B, DF, seq]
    B, _, seq = a_t.shape
    a16 = a_t.astype(np.float16)
    a16s = (a16.astype(np.float32) * 32.0).astype(np.float16)
    alo8 = ((a_t - a16.astype(np.float32)) * SC).astype(ml_dtypes.float8_e4m3)
    a8 = a_t.astype(ml_dtypes.float8_e4m3)
    a8pair = np.empty((B, P, 2, 2, seq), dtype=ml_dtypes.float8_e4m3)
    a8pair[:, :, :, 0, :] = alo8.reshape(B, 2, P, seq).transpose(0, 2, 1, 3)
    a8pair[:, :, :, 1, :] = a8.reshape(B, 2, P, seq).transpose(0, 2, 1, 3)
    a16v = np.ascontiguousarray(a16s.reshape(B, 2, P, seq))

    # host-side weight prep (exact fp32)
    mapped_b = np.einsum("bjf,fh->bjh", input_b, Wb) + bb       # [B, 256, 64]
    wf = 8.0 * np.einsum("fh,bjh->bfj", Wa, mapped_b)           # [B, 256(f), 256(j)]
    w16 = wf.astype(np.float16).astype(np.float32)
    w16s = (w16 * 64.0).astype(np.float16)                      # fp16(Wf)*64
    wlo8 = ((wf - w16) * SC).astype(ml_dtypes.float8_e4m3)
    w8 = wf.astype(ml_dtypes.float8_e4m3)
    # [B, 128(f%128), 2(fchunk), 2(w8|wlo8), 256(j)]
    w8pair = np.empty((B, P, 2, 2, DF), dtype=ml_dtypes.float8_e4m3)
    w8pair[:, :, :, 0, :] = w8.reshape(B, 2, P, DF).transpose(0, 2, 1, 3)
    w8pair[:, :, :, 1, :] = wlo8.reshape(B, 2, P, DF).transpose(0, 2, 1, 3)
    w16v = np.ascontiguousarray(
        w16s.reshape(B, 2, P, DF).transpose(0, 2, 1, 3))        # [B, 128, 2, 256]

    wout = np.einsum("bjh,hf->bjf", mapped_b, Wc) + bc          # [B, 256(j), 256(f)]
    wo16 = np.ascontiguousarray(
        wout.astype(np.float16).reshape(B, 2, P, DF).transpose(0, 2, 1, 3))
    sbias = (8.0 * SC) * np.einsum("h,bjh->bj", ba, mapped_b)   # [B, 256(j)]
    sbias = np.ascontiguousarray(sbias.reshape(B, 1, DF).astype(np.float32))

    shared = {
        "eye_d": np.eye(P, dtype=np.float16),
        "ones_d": np.ones((1, P), dtype=np.float32),
    }
    in_maps = []
    for c in range(n_cores):
        lo, hi = c * batches, (c + 1) * batches
        in_maps.append({
            "a16_d": np.ascontiguousarray(a16v[lo:hi]),
            "a8_d": np.ascontiguousarray(a8pair[lo:hi]),
            "w16_d": np.ascontiguousarray(w16v[lo:hi]),
            "w8_d": np.ascontiguousarray(w8pair[lo:hi]),
            "wo_d": np.ascontiguousarray(wo16[lo:hi]),
            "sb_d": np.ascontiguousarray(sbias[lo:hi]),
            **shared,
        })
    return in_maps


def _postprocess(results, seq=SEQ, batches=BATCHES_PER_CORE):
    """Concatenate per-core outputs, transpose, and normalize by sumexp."""
    outs = np.concatenate(
        [np.asarray(r["out_t"], dtype=np.float32) for r in results], axis=0)
    sums = np.concatenate(
        [np.asarray(r["sum_d"], dtype=np.float32) for r in results], axis=0)
    B = outs.shape[0]
    # sums[b, p, 4*gm + s] -> row i = 512*gm + 128*s + p
    n_cm = sums.shape[2] // 4
    grid = sums.reshape(B, P, n_cm, 4).transpose(0, 2, 3, 1).reshape(B, -1)
    se = grid[:, :seq]                                  # [B, seq]
    out = outs.transpose(0, 2, 1) / se[:, :, None]
    return np.ascontiguousarray(out.astype(np.float32))


def kernel(input_a, input_b, Wa, ba, Wb, bb, Wc, bc):
    use_ba = bool(np.any(np.asarray(ba)))
    nc = _get_program(use_ba=use_ba)
    in_maps = make_in_maps(input_a, input_b, Wa, ba, Wb, bb, Wc, bc)
    res = run_bass_kernel_spmd(nc, in_maps, core_ids=list(range(N_CORES)))
    return _postprocess(res.results)
